# revision 1
# baseline (speedup 1.0000x reference)
"""Trainium2 Bass kernel for nn_LlamaMLP_HalfwayGIN_MultiAggregration.

Sharding: 16 heads -> 8 cores (2 heads/core). Each core computes its two
heads' full pipeline plus the partial down-projection; host sums partials.

Per-core dataflow is fully "transposed" (d on partitions for aggregates):
  h   = silu(x@Wg.T)*(x@Wu.T)            s-major  [s=2048, 512]
  hT  = per-head PE transpose            d-major  [256, 2048] x2
  QT/KT = Wq,Wk projections              e-major  [256, 2048]
  per (head, s-window 512): stream adjT t-chunks once, accumulate
      scoresT = KT.T@QT, E = exp(scoresT)*adjT
      sum_aggT += h.T@adjT ; attn_numT += h.T@E ; denom += ones.T@E
  attn_aggT = attn_numT * broadcast(1/denom)
  y1T = silu(W1ac.T@hT + W1b.T@sum_aggT + W1d.T@attn_aggT)
  ginT = W2T.T@y1T ;  out_partial = ginT.T @ WdT_local
Folds on host: (1+eps),alpha into W1 blocks; 1/sqrt(D) into Wq.
"""

import math
import os
import numpy as np
import ml_dtypes

B, S, HID, NH, INTER = 1, 2048, 1024, 16, 4096
D = 256
NCORES = 8
HPC = NH // NCORES          # 2 heads per core
LOC = HPC * D               # 512 local intermediate dims
BF16 = ml_dtypes.bfloat16

_CACHE = {}


def _build_nc():
    import concourse.mybir as mybir
    import concourse.tile as tile
    from concourse import bacc
    from concourse.masks import make_identity
    from contextlib import ExitStack

    f32 = mybir.dt.float32
    bf16 = mybir.dt.bfloat16
    fp8 = mybir.dt.float8e4
    AF = mybir.ActivationFunctionType

    nc = bacc.Bacc("TRN2", target_bir_lowering=False, debug=False)

    xT_d = nc.dram_tensor("xT", [HID, S], bf16, kind="ExternalInput")
    wg_d = nc.dram_tensor("wgT", [HID, LOC], bf16, kind="ExternalInput")
    wu_d = nc.dram_tensor("wuT", [HID, LOC], bf16, kind="ExternalInput")
    adj_d = nc.dram_tensor("adjT", [HPC, S, S], bf16, kind="ExternalInput")
    wq_d = nc.dram_tensor("wqT", [HPC, D, D], fp8, kind="ExternalInput")
    wk_d = nc.dram_tensor("wkT", [HPC, D, D], fp8, kind="ExternalInput")
    w1ac_d = nc.dram_tensor("w1acT", [HPC, D, D], bf16, kind="ExternalInput")
    w1b_d = nc.dram_tensor("w1bT", [HPC, D, D], bf16, kind="ExternalInput")
    w1d_d = nc.dram_tensor("w1dT", [D, D], bf16, kind="ExternalInput")
    w2_d = nc.dram_tensor("w2T", [D, D], bf16, kind="ExternalInput")
    wd_d = nc.dram_tensor("wdT", [LOC, HID], bf16, kind="ExternalInput")
    out_d = nc.dram_tensor("out", [S, HID], f32, kind="ExternalOutput")

    NST = S // 128            # 16 s-tiles
    NSW = S // 512            # 4 s-windows
    NTC = S // 128            # 16 t-chunks
    NKC = HID // 128          # 8 k-chunks

    with ExitStack() as es:
        tc = es.enter_context(tile.TileContext(nc))

        persist = es.enter_context(tc.tile_pool(name="persist", bufs=1))
        h_all = persist.tile([128, NST, LOC], bf16, name="h_all")
        hT_all = persist.tile([128, 2 * HPC, S], bf16, name="hT_all")
        hT8 = persist.tile([128, 2 * HPC, S], fp8, name="hT8")
        ginT_all = persist.tile([128, 2 * HPC, S], bf16, name="ginT_all")

        wpool = es.enter_context(tc.tile_pool(name="weights", bufs=1))
        wq_sb = wpool.tile([128, 2 * HPC, D], fp8, name="wq_sb")
        wk_sb = wpool.tile([128, 2 * HPC, D], fp8, name="wk_sb")
        w1ac_sb = wpool.tile([128, 2 * HPC, D], bf16, name="w1ac_sb")
        w1b_sb = wpool.tile([128, 2 * HPC, D], bf16, name="w1b_sb")
        w1d_sb = wpool.tile([128, 2, D], bf16, name="w1d_sb")
        w2_sb = wpool.tile([128, 2, D], bf16, name="w2_sb")
        wd_sb = wpool.tile([128, LOC // 128, HID], bf16, name="wd_sb")

        misc = es.enter_context(tc.tile_pool(name="misc", bufs=1))
        id_sb = misc.tile([128, 128], bf16, name="id_sb")
        ones128 = misc.tile([128, 1], bf16, name="ones128")
        ones1 = misc.tile([1, 128], bf16, name="ones1")

        make_identity(nc, id_sb)
        nc.vector.memset(ones128, 1.0)
        nc.vector.memset(ones1, 1.0)

        # weight loads
        nc.scalar.dma_start(wq_sb, wq_d.rearrange("h (c p) e -> p (h c) e", p=128))
        nc.scalar.dma_start(wk_sb, wk_d.rearrange("h (c p) e -> p (h c) e", p=128))
        nc.scalar.dma_start(w1ac_sb, w1ac_d.rearrange("h (c p) o -> p (h c) o", p=128))
        nc.scalar.dma_start(w1b_sb, w1b_d.rearrange("h (c p) o -> p (h c) o", p=128))
        nc.scalar.dma_start(w1d_sb, w1d_d.rearrange("(c p) o -> p c o", p=128))
        nc.scalar.dma_start(w2_sb, w2_d.rearrange("(c p) o -> p c o", p=128))
        nc.scalar.dma_start(wd_sb, wd_d.rearrange("(c p) o -> p c o", p=128))

        # ---- phase 1: h = silu(x@WgT)*(x@WuT), then hT via PE transpose ----
        with tc.tile_pool(name="xpool", bufs=1) as xpool, \
             tc.tile_pool(name="ps1", bufs=1, space="PSUM") as ps1, \
             tc.tile_pool(name="hstage", bufs=3) as hstage:
            xT_sb = xpool.tile([128, NKC, S], bf16, name="xT_sb")
            wg_sb = xpool.tile([128, NKC, LOC], bf16, name="wg_sb")
            wu_sb = xpool.tile([128, NKC, LOC], bf16, name="wu_sb")
            # split the big loads across DMA queues so the h-phase isn't
            # gated on one serial transfer
            xT_re = xT_d.rearrange("(c p) s -> p c s", p=128)
            wg_re = wg_d.rearrange("(c p) o -> p c o", p=128)
            wu_re = wu_d.rearrange("(c p) o -> p c o", p=128)
            # per-chunk interleaved loads: chunk 0 of xT/wg lands first so
            # the first matmul starts early; ~24 issues stay cheap on Sync
            for c in range(NKC):
                nc.sync.dma_start(xT_sb[:, c, :], xT_re[:, c, :])
                nc.sync.dma_start(wg_sb[:, c, :], wg_re[:, c, :])
                nc.sync.dma_start(wu_sb[:, c, :], wu_re[:, c, :])

            for st in range(NST):
                g_ps = ps1.tile([128, LOC], f32, name=f"g{st}", tag="g", bufs=2)
                u_ps = ps1.tile([128, LOC], f32, name=f"u{st}", tag="u", bufs=2)
                for c in range(NKC):
                    lhsT = xT_sb[:, c, st * 128:(st + 1) * 128]
                    nc.tensor.matmul(g_ps, lhsT, wg_sb[:, c, :],
                                     start=(c == 0), stop=(c == NKC - 1))
                    nc.tensor.matmul(u_ps, lhsT, wu_sb[:, c, :],
                                     start=(c == 0), stop=(c == NKC - 1))
                sg = hstage.tile([128, LOC], bf16, name=f"sg{st}", tag="sg")
                nc.scalar.activation(sg, g_ps, AF.Silu)
                nc.vector.tensor_mul(h_all[:, st, :], sg, u_ps)

                # transpose this s-tile's four d-chunks right away so hT
                # lands incrementally while later h tiles still stream
                tr_ps = ps1.tile([128, 4, 128], bf16, name=f"tr{st}",
                                 tag="tr", bufs=2)
                for j in range(2 * HPC):
                    hd, dc = j // 2, j % 2
                    col0 = hd * D + dc * 128
                    nc.tensor.transpose(tr_ps[:, j, :],
                                        h_all[:, st, col0:col0 + 128], id_sb)
                stsl = slice(st * 128, (st + 1) * 128)
                nc.vector.tensor_copy(hT_all[:, :, stsl], tr_ps)
                nc.vector.tensor_copy(hT8[:, :, stsl], tr_ps)

        # ---- phase 2: attention (both heads) ----
        with tc.tile_pool(name="perhead", bufs=2) as php, \
             tc.tile_pool(name="stream", bufs=1) as strm, \
             tc.tile_pool(name="outp", bufs=2) as outp:
            heads = []
            with tc.tile_pool(name="ps2", bufs=1, space="PSUM") as ps2:
                for hd in range(HPC):
                    qT = php.tile([128, 2, S], fp8, name=f"qT{hd}", tag="qT")
                    kT = php.tile([128, 2, S], fp8, name=f"kT{hd}", tag="kT")
                    sumT = php.tile([128, 2, S], bf16, name=f"sumT{hd}", tag="sumT")
                    attnT = php.tile([128, 2, S], bf16, name=f"attnT{hd}", tag="attnT")
                    y1T = php.tile([128, 2, S], bf16, name=f"y1T{hd}", tag="y1T")
                    heads.append((qT, kT, sumT, attnT, y1T))

                    # QK projections (fp8 DoubleRow over both d-chunks; the
                    # 32x fp8 range scale is folded into wq/wk host-side)
                    for w_sb, dstT in ((wq_sb, qT), (wk_sb, kT)):
                        for et in range(2):
                            for sw in range(NSW):
                                ssl = slice(sw * 512, (sw + 1) * 512)
                                ps = ps2.tile([128, 512], f32,
                                              name=f"qk{hd}_{et}_{sw}", tag="mm512",
                                              bufs=3)
                                nc.tensor.matmul(
                                    ps,
                                    w_sb[:, hd * 2:hd * 2 + 2, et * 128:(et + 1) * 128],
                                    hT8[:, hd * 2:hd * 2 + 2, ssl],
                                    start=True, stop=True,
                                    perf_mode=mybir.MatmulPerfMode.DoubleRow)
                                nc.vector.tensor_copy(dstT[:, et, ssl], ps)

                    for sw in range(NSW):
                        ssl = slice(sw * 512, (sw + 1) * 512)
                        sum_ps = ps2.tile([128, 2, 512], f32,
                                          name=f"sum{hd}_{sw}", tag="sum")
                        att_ps = ps2.tile([128, 2, 512], f32,
                                          name=f"att{hd}_{sw}", tag="att")
                        den_ps = ps2.tile([1, 512], f32,
                                          name=f"den{hd}_{sw}", tag="den")
                        # one-iteration software pipeline: scores(t) issue, then
                        # the paired sum/att/den for t-1 — pairs share a
                        # stationary h lhsT and the exp+mask latency is hidden
                        em_tiles = {}
                        adj_tiles = {}
                        for tcx in range(NTC + 1):
                            if tcx < NTC:
                                tsl = slice(tcx * 128, (tcx + 1) * 128)
                                adj_t = strm.tile([128, 512], bf16,
                                                  name=f"adj{hd}_{sw}_{tcx}",
                                                  tag="adj", bufs=10)
                                nc.sync.dma_start(adj_t, adj_d[hd, tsl, ssl])
                                adj_tiles[tcx] = adj_t
                                sc_ps = ps2.tile([128, 512], f32,
                                                 name=f"sc{hd}_{sw}_{tcx}",
                                                 tag="mm512", bufs=3)
                                nc.tensor.matmul(
                                    sc_ps, kT[:, :, tsl], qT[:, :, ssl],
                                    start=True, stop=True,
                                    perf_mode=mybir.MatmulPerfMode.DoubleRow)
                                em_t = strm.tile([128, 512], bf16,
                                                 name=f"em{hd}_{sw}_{tcx}",
                                                 tag="em", bufs=6)
                                nc.scalar.activation(em_t, sc_ps, AF.Exp,
                                                     scale=1.0 / 1024.0)
                                nc.vector.tensor_mul(em_t, em_t, adj_t)
                                em_tiles[tcx] = em_t
                            if tcx >= 1:
                                p = tcx - 1
                                em_p = em_tiles.pop(p)
                                adj_p = adj_tiles.pop(p)
                                first, last = p == 0, p == NTC - 1
                                for dc in range(2):
                                    h_lhsT = h_all[:, p,
                                                   hd * D + dc * 128:hd * D + (dc + 1) * 128]
                                    nc.tensor.matmul(sum_ps[:, dc, :], h_lhsT, adj_p,
                                                     start=first, stop=last)
                                    nc.tensor.matmul(att_ps[:, dc, :], h_lhsT, em_p,
                                                     start=first, stop=last)
                                nc.tensor.matmul(den_ps, ones128, em_p,
                                                 start=first, stop=last)

                        # evictions
                        nc.vector.tensor_copy(sumT[:, :, ssl], sum_ps)
                        r32 = strm.tile([1, 512], f32, name=f"r32_{hd}_{sw}",
                                        tag="r32", bufs=2)
                        nc.vector.reciprocal_approx_fast(r32, den_ps)
                        rbf = strm.tile([1, 512], bf16, name=f"rbf_{hd}_{sw}",
                                        tag="rbf", bufs=2)
                        nc.vector.tensor_copy(rbf, r32)
                        rb_ps = ps2.tile([128, 512], f32, name=f"rb{hd}_{sw}",
                                         tag="mm512", bufs=3)
                        nc.tensor.matmul(rb_ps, ones1, rbf, start=True, stop=True)
                        rb_sb = strm.tile([128, 512], bf16, name=f"rbsb{hd}_{sw}",
                                          tag="rbsb", bufs=2)
                        nc.vector.tensor_copy(rb_sb, rb_ps)
                        for dc in range(2):
                            nc.vector.tensor_mul(attnT[:, dc, ssl],
                                                 att_ps[:, dc, :], rb_sb)

            # ---- phase 3: GIN MLP + partial down-projection ----
            with tc.tile_pool(name="ps3", bufs=1, space="PSUM") as ps3:
                for sw in range(NSW):
                    ssl = slice(sw * 512, (sw + 1) * 512)
                    for hd in range(HPC):
                        qT, kT, sumT, attnT, y1T = heads[hd]
                        for ot in range(2):
                            osl = slice(ot * 128, (ot + 1) * 128)
                            y1_ps = ps3.tile([128, 512], f32,
                                             name=f"y1{hd}_{sw}_{ot}", tag="mmout",
                                             bufs=6)
                            kk = 0
                            for w_sb, rhs_of in (
                                (w1ac_sb, lambda dc: hT_all[:, hd * 2 + dc, ssl]),
                                (w1b_sb, lambda dc: sumT[:, dc, ssl]),
                                (None, lambda dc: attnT[:, dc, ssl]),
                            ):
                                for dc in range(2):
                                    if w_sb is None:
                                        lhsT = w1d_sb[:, dc, osl]
                                    else:
                                        lhsT = w_sb[:, hd * 2 + dc, osl]
                                    nc.tensor.matmul(y1_ps, lhsT, rhs_of(dc),
                                                     start=(kk == 0), stop=(kk == 5))
                                    kk += 1
                            nc.scalar.activation(y1T[:, ot, ssl], y1_ps, AF.Silu)
                        for ot in range(2):
                            osl = slice(ot * 128, (ot + 1) * 128)
                            gin_ps = ps3.tile([128, 512], f32,
                                              name=f"gin{hd}_{sw}_{ot}", tag="mmout",
                                              bufs=6)
                            for dc in range(2):
                                nc.tensor.matmul(gin_ps, w2_sb[:, dc, osl],
                                                 y1T[:, dc, ssl],
                                                 start=(dc == 0), stop=(dc == 1))
                            nc.vector.tensor_copy(ginT_all[:, hd * 2 + ot, ssl],
                                                  gin_ps)
                    # down-projection for this window's four s-tiles
                    for st in range(sw * 4, sw * 4 + 4):
                        stsl = slice(st * 128, (st + 1) * 128)
                        o_sb = outp.tile([128, HID], f32, name=f"o_sb{st}",
                                         tag="o_sb")
                        for nw in range(2):
                            d_ps = ps3.tile([128, 512], f32, name=f"d{st}_{nw}",
                                            tag="mmout", bufs=6)
                            for j in range(LOC // 128):
                                nc.tensor.matmul(d_ps, ginT_all[:, j, stsl],
                                                 wd_sb[:, j, nw * 512:(nw + 1) * 512],
                                                 start=(j == 0),
                                                 stop=(j == LOC // 128 - 1))
                            nc.vector.tensor_copy(o_sb[:, nw * 512:(nw + 1) * 512],
                                                  d_ps)
                        nc.gpsimd.dma_start(out_d[stsl, :], o_sb)

    nc.compile()
    return nc


def _prep_in_maps(x, adjacency, Wg, Wu, Wd, eps, alpha, Wq, Wk, W1, W2):
    f = lambda a: np.ascontiguousarray(a, dtype=np.float32)
    x, adjacency = f(x), f(adjacency)
    Wg, Wu, Wd, Wq, Wk, W1, W2 = map(f, (Wg, Wu, Wd, Wq, Wk, W1, W2))
    eps, alpha = f(eps), f(alpha)
    b16 = lambda a: np.ascontiguousarray(a).astype(BF16)

    xT = b16(x[0].T)                                  # (HID, S)
    # fp8 Q/K path: q' = 8*Q, k' = 8*K -> psum = 64*QK; the kernel's exp
    # applies scale 1/1024 = 1/(64*sqrt(D))
    f8 = lambda a: np.ascontiguousarray(a).astype(ml_dtypes.float8_e4m3)
    in_maps = []
    for i in range(NCORES):
        hs = slice(i * HPC, (i + 1) * HPC)
        c0, c1 = i * LOC, (i + 1) * LOC
        W1a = W1[:, 0:D]
        W1b = W1[:, D:2 * D]
        W1c = W1[:, 2 * D:3 * D]
        W1d = W1[:, 3 * D:4 * D]
        w1ac = np.stack([((1.0 + eps[h]) * W1a + W1c).T
                         for h in range(i * HPC, (i + 1) * HPC)])
        w1b = np.stack([(alpha[h] * W1b).T
                        for h in range(i * HPC, (i + 1) * HPC)])
        in_maps.append({
            "xT": xT,
            "wgT": b16(Wg[c0:c1].T),
            "wuT": b16(Wu[c0:c1].T),
            "adjT": b16(adjacency[0, hs].transpose(0, 2, 1)),
            "wqT": f8(Wq[hs].transpose(0, 2, 1) * 8.0),
            "wkT": f8(Wk[hs].transpose(0, 2, 1) * 8.0),
            "w1acT": b16(w1ac),
            "w1bT": b16(w1b),
            "w1dT": b16(W1d.T),
            "w2T": b16(W2.T),
            "wdT": b16(Wd[:, c0:c1].T),
        })
    return in_maps


def _run(inputs, trace=False, trace_kwargs=None):
    from concourse.bass_utils import run_bass_kernel_spmd

    if "nc" not in _CACHE:
        _CACHE["nc"] = _build_nc()
    nc = _CACHE["nc"]
    in_maps = _prep_in_maps(**inputs)
    res = run_bass_kernel_spmd(nc, in_maps, list(range(NCORES)),
                               trace=trace, **(trace_kwargs or {}))
    out = np.zeros((S, HID), np.float32)
    for r in res.results:
        out += r["out"]
    return out.reshape(B, S, HID), res


def kernel(**inputs) -> np.ndarray:
    out, _ = _run(inputs, trace=False)
    return out



# revision 5
# speedup vs baseline: 1.1331x; 1.1331x over previous
"""Trainium2 Bass kernel for nn_LlamaMLP_HalfwayGIN_MultiAggregration.

Sharding: 16 heads -> 8 cores (2 heads/core). Each core computes its two
heads' full pipeline plus the partial down-projection; host sums partials.

v2: fp8 DoubleRow for the attention-aggregation + denominator matmuls
(t-pairs, contraction 256), fp8 DR for the small y1 terms, contiguous
host-side DMA layouts (one 2MB DMA per adjacency stripe), sw-outer loop
with the GIN MLP + down-projection interleaved per s-window.

Scale folding (host):
  hT8/h8 = 8*h (fp8); wq8 = 8*Wq^T, wk8 = 8*Wk^T  -> scores psum = 4096*QK^T
  exp scale = 1/(4096*sqrt(D)) = 1/65536
  adj16 = 16*adjacency^T (bf16)  -> em8 = 16*em, sumT = 16*sum_raw
  w1b = 32*alpha*W1b^T (bf16: 512/16), w1ac8 = 64*((1+eps)W1a+W1c)^T (fp8:
  512/8 vs h8), w1d8 = 64*W1d^T (fp8: 512/8 vs attnT=8*attn), silu scale 1/512
"""

import math
import os
import numpy as np
import ml_dtypes

B, S, HID, NH, INTER = 1, 2048, 1024, 16, 4096
D = 256
NCORES = 8
HPC = NH // NCORES          # 2 heads per core
LOC = HPC * D               # 512 local intermediate dims
BF16 = ml_dtypes.bfloat16
FP8 = ml_dtypes.float8_e4m3

_CACHE = {}


def _build_nc():
    import concourse.mybir as mybir
    import concourse.tile as tile
    from concourse import bacc
    from concourse.masks import make_identity
    from contextlib import ExitStack

    f32 = mybir.dt.float32
    bf16 = mybir.dt.bfloat16
    fp8 = mybir.dt.float8e4
    AF = mybir.ActivationFunctionType
    DR = mybir.MatmulPerfMode.DoubleRow

    nc = bacc.Bacc("TRN2", target_bir_lowering=False, debug=False)

    NST = S // 128            # 16 s-tiles
    NSW = S // 512            # 4 s-windows
    NTC = S // 128            # 16 t-chunks
    NPR = NTC // 2            # 8 t-pairs
    NKC = HID // 128          # 8 k-chunks

    x_d = nc.dram_tensor("xb", [NST, 128, NKC, 128], bf16, kind="ExternalInput")
    wg_d = nc.dram_tensor("wgb", [NKC, 128, LOC], bf16, kind="ExternalInput")
    wu_d = nc.dram_tensor("wub", [NKC, 128, LOC], bf16, kind="ExternalInput")
    adj_d = nc.dram_tensor("adjb", [NSW, HPC, 128, NTC, 512], bf16,
                           kind="ExternalInput")
    wq_d = nc.dram_tensor("wqT", [HPC, D, D], fp8, kind="ExternalInput")
    wk_d = nc.dram_tensor("wkT", [HPC, D, D], fp8, kind="ExternalInput")
    w1ac_d = nc.dram_tensor("w1acT", [HPC, D, D], fp8, kind="ExternalInput")
    w1b_d = nc.dram_tensor("w1bT", [HPC, D, D], bf16, kind="ExternalInput")
    w1d_d = nc.dram_tensor("w1dT", [D, D], fp8, kind="ExternalInput")
    w2_d = nc.dram_tensor("w2T", [D, D], bf16, kind="ExternalInput")
    wd_d = nc.dram_tensor("wdT", [LOC, HID], bf16, kind="ExternalInput")
    out_d = nc.dram_tensor("out", [S, HID], f32, kind="ExternalOutput")

    with ExitStack() as es:
        tc = es.enter_context(tile.TileContext(nc))

        persist = es.enter_context(tc.tile_pool(name="persist", bufs=1))
        h_all = persist.tile([128, NST, LOC], bf16, name="h_all")
        h8_all = persist.tile([128, NST, LOC], fp8, name="h8_all")
        hT8 = persist.tile([128, 2 * HPC, S], fp8, name="hT8")
        qT = persist.tile([128, HPC, 2, S], fp8, name="qT")
        kT = persist.tile([128, HPC, 2, S], fp8, name="kT")

        wpool = es.enter_context(tc.tile_pool(name="weights", bufs=1))
        wq_sb = wpool.tile([128, 2 * HPC, D], fp8, name="wq_sb")
        wk_sb = wpool.tile([128, 2 * HPC, D], fp8, name="wk_sb")
        w1ac_sb = wpool.tile([128, 2 * HPC, D], fp8, name="w1ac_sb")
        w1b_sb = wpool.tile([128, 2 * HPC, D], bf16, name="w1b_sb")
        w1d_sb = wpool.tile([128, 2, D], fp8, name="w1d_sb")
        w2_sb = wpool.tile([128, 2, D], bf16, name="w2_sb")
        wd_sb = wpool.tile([128, LOC // 128, HID], bf16, name="wd_sb")

        misc = es.enter_context(tc.tile_pool(name="misc", bufs=1))
        id_sb = misc.tile([128, 128], bf16, name="id_sb")
        ones2 = misc.tile([128, 2, 16], fp8, name="ones2")

        make_identity(nc, id_sb)
        nc.vector.memset(ones2, 1.0)

        # small weight loads on the sync queue (issued after wg/wu below
        # would be ideal, but sync order is fine: ~1.2MB total)
        nc.sync.dma_start(wq_sb, wq_d.rearrange("h (c p) e -> p (h c) e", p=128))
        nc.sync.dma_start(wk_sb, wk_d.rearrange("h (c p) e -> p (h c) e", p=128))
        nc.sync.dma_start(w1ac_sb, w1ac_d.rearrange("h (c p) o -> p (h c) o", p=128))
        nc.sync.dma_start(w1b_sb, w1b_d.rearrange("h (c p) o -> p (h c) o", p=128))
        nc.sync.dma_start(w1d_sb, w1d_d.rearrange("(c p) o -> p c o", p=128))
        nc.sync.dma_start(w2_sb, w2_d.rearrange("(c p) o -> p c o", p=128))
        nc.sync.dma_start(wd_sb, wd_d.rearrange("(c p) o -> p c o", p=128))

        # adjacency stripes: one contiguous 2MB DMA per (sw, hd), deep
        # prefetch via pool depth; issued up-front on the gpsimd queue
        adjpool = es.enter_context(tc.tile_pool(name="adj", bufs=1))
        adj_tiles = {}
        for sw in range(NSW):
            for hd in range(HPC):
                a = adjpool.tile([128, NTC, 512], bf16,
                                 name=f"adj{sw}_{hd}", tag="adj", bufs=3)
                nc.gpsimd.dma_start(a, adj_d[sw, hd])
                adj_tiles[(sw, hd)] = a

        # ---- phase 1: h = silu(x@WgT)*(x@WuT); hT8, h8 side copies ----
        with tc.tile_pool(name="xpool", bufs=1) as xpool, \
             tc.tile_pool(name="ps1", bufs=1, space="PSUM") as ps1, \
             tc.tile_pool(name="hstage", bufs=3) as hstage:
            x_sb = xpool.tile([128, NST, NKC, 128], bf16, name="x_sb")
            wg_sb = xpool.tile([128, NKC, LOC], bf16, name="wg_sb")
            wu_sb = xpool.tile([128, NKC, LOC], bf16, name="wu_sb")
            # wg/wu chunk-interleaved on sync queue; x st-blocks on scalar
            for c in range(NKC):
                nc.sync.dma_start(wg_sb[:, c, :], wg_d[c])
                nc.sync.dma_start(wu_sb[:, c, :], wu_d[c])
            for st in range(NST):
                nc.scalar.dma_start(x_sb[:, st], x_d[st])

            def do_tr(st):
                # transpose s-tile st's four d-chunks (pipelined one behind)
                tr_ps = ps1.tile([128, 2 * HPC, 128], bf16, name=f"tr{st}",
                                 tag="tr", bufs=2)
                for j in range(2 * HPC):
                    col0 = j * 128
                    nc.tensor.transpose(tr_ps[:, j, :],
                                        h_all[:, st, col0:col0 + 128], id_sb)
                stsl = slice(st * 128, (st + 1) * 128)
                nc.vector.tensor_scalar_mul(hT8[:, :, stsl], tr_ps, 8.0)

            for st in range(NST):
                g_ps = ps1.tile([128, LOC], f32, name=f"g{st}", tag="g", bufs=2)
                u_ps = ps1.tile([128, LOC], f32, name=f"u{st}", tag="u", bufs=2)
                for c in range(NKC):
                    lhsT = x_sb[:, st, c, :]
                    nc.tensor.matmul(g_ps, lhsT, wg_sb[:, c, :],
                                     start=(c == 0), stop=(c == NKC - 1))
                    nc.tensor.matmul(u_ps, lhsT, wu_sb[:, c, :],
                                     start=(c == 0), stop=(c == NKC - 1))
                if st >= 1:
                    do_tr(st - 1)
                sg = hstage.tile([128, LOC], bf16, name=f"sg{st}", tag="sg")
                nc.scalar.activation(sg, g_ps, AF.Silu)
                nc.vector.tensor_mul(h_all[:, st, :], sg, u_ps)
                nc.vector.tensor_scalar_mul(h8_all[:, st, :],
                                            h_all[:, st, :], 8.0)
            do_tr(NST - 1)

            # QK projections for both heads (fp8 DoubleRow, contraction 256)
            for hd in range(HPC):
                for w_sb, dstT in ((wq_sb, qT), (wk_sb, kT)):
                    for et in range(2):
                        for sw in range(NSW):
                            ssl = slice(sw * 512, (sw + 1) * 512)
                            ps = ps1.tile([128, 512], f32,
                                          name=f"qk{hd}_{et}_{sw}", tag="g",
                                          bufs=2)
                            nc.tensor.matmul(
                                ps,
                                w_sb[:, hd * 2:hd * 2 + 2, et * 128:(et + 1) * 128],
                                hT8[:, hd * 2:hd * 2 + 2, ssl],
                                start=True, stop=True, perf_mode=DR)
                            nc.vector.tensor_copy(dstT[:, hd, et, ssl], ps)

        # ---- phase 2+3 fused, sw-outer ----
        with tc.tile_pool(name="stream", bufs=1) as strm, \
             tc.tile_pool(name="outp", bufs=2) as outp, \
             tc.tile_pool(name="ps2", bufs=1, space="PSUM") as ps2:
            for sw in range(NSW):
                ssl = slice(sw * 512, (sw + 1) * 512)
                hd_res = []
                for hd in range(HPC):
                    adj_sb = adj_tiles[(sw, hd)]
                    sum_ps = ps2.tile([128, 2, 512], f32,
                                      name=f"sum{sw}_{hd}", tag="sum")
                    att_ps = ps2.tile([128, 2, 512], f32,
                                      name=f"att{sw}_{hd}", tag="att")
                    den_ps = ps2.tile([1, 512], f32,
                                      name=f"den{sw}_{hd}", tag="den", bufs=1)
                    # one-pair software pipeline: issue scores/exp/em for
                    # pair pr while the matmuls consume pair pr-1
                    em_tiles = {}
                    for pr in range(NPR + 1):
                        if pr < NPR:
                            em8 = strm.tile([128, 2, 512], fp8,
                                            name=f"em{sw}_{hd}_{pr}",
                                            tag="em", bufs=4)
                            for i in range(2):
                                t = 2 * pr + i
                                tsl = slice(t * 128, (t + 1) * 128)
                                sc_ps = ps2.tile([128, 512], f32,
                                                 name=f"sc{sw}_{hd}_{pr}_{i}",
                                                 tag="mm", bufs=3)
                                nc.tensor.matmul(sc_ps, kT[:, hd, :, tsl],
                                                 qT[:, hd, :, ssl],
                                                 start=True, stop=True,
                                                 perf_mode=DR)
                                ex = strm.tile([128, 512], bf16,
                                               name=f"ex{sw}_{hd}_{pr}_{i}",
                                               tag="ex", bufs=4)
                                nc.scalar.activation(ex, sc_ps, AF.Exp,
                                                     scale=1.0 / 65536.0)
                                nc.vector.tensor_mul(em8[:, i, :], ex,
                                                     adj_sb[:, t, :])
                            em_tiles[pr] = em8
                        if pr >= 1:
                            p = pr - 1
                            em_p = em_tiles.pop(p)
                            first, last = p == 0, p == NPR - 1
                            for dc in range(2):
                                c0 = hd * D + dc * 128
                                for i in range(2):
                                    t = 2 * p + i
                                    nc.tensor.matmul(
                                        sum_ps[:, dc, :],
                                        h_all[:, t, c0:c0 + 128],
                                        adj_sb[:, t, :],
                                        start=(first and i == 0),
                                        stop=(last and i == 1))
                                nc.tensor.matmul(
                                    att_ps[:, dc, :],
                                    h8_all[:, 2 * p:2 * p + 2, c0:c0 + 128],
                                    em_p, start=first, stop=last,
                                    perf_mode=DR)
                            nc.tensor.matmul(den_ps, ones2[:, :, 0:1], em_p,
                                             start=first, stop=last,
                                             perf_mode=DR)

                    # evictions: sumT (vector), attnT = att/den (gpsimd bcast
                    # + vector mul), all into per-iteration tiles
                    sumT = strm.tile([128, 2, 512], bf16, name=f"sumT{sw}_{hd}",
                                     tag="sumT", bufs=2)
                    nc.vector.tensor_copy(sumT, sum_ps)
                    r32 = strm.tile([1, 512], f32, name=f"r32_{sw}_{hd}",
                                    tag="r32", bufs=2)
                    nc.vector.reciprocal_approx_fast(r32, den_ps)
                    rbf = strm.tile([1, 512], bf16, name=f"rbf_{sw}_{hd}",
                                    tag="rbf", bufs=2)
                    nc.vector.tensor_copy(rbf, r32)
                    rb_sb = strm.tile([128, 512], bf16, name=f"rb{sw}_{hd}",
                                      tag="rb", bufs=2)
                    nc.gpsimd.partition_broadcast(rb_sb, rbf)
                    attnT = strm.tile([128, 2, 512], fp8, name=f"at{sw}_{hd}",
                                      tag="attnT", bufs=2)
                    for dc in range(2):
                        nc.vector.tensor_mul(attnT[:, dc, :],
                                             att_ps[:, dc, :], rb_sb)
                    hd_res.append((sumT, attnT))

                # GIN MLP for both heads at this window
                ginT = strm.tile([128, 2 * HPC, 512], bf16, name=f"gin{sw}",
                                 tag="gin", bufs=2)
                for hd in range(HPC):
                    sumT, attnT = hd_res[hd]
                    y1T = strm.tile([128, 2, 512], bf16, name=f"y1{sw}_{hd}",
                                    tag="y1", bufs=2)
                    for ot in range(2):
                        osl = slice(ot * 128, (ot + 1) * 128)
                        y1_ps = ps2.tile([128, 512], f32,
                                         name=f"y1p{sw}_{hd}_{ot}", tag="mm",
                                         bufs=3)
                        for dc in range(2):
                            nc.tensor.matmul(y1_ps,
                                             w1b_sb[:, hd * 2 + dc, osl],
                                             sumT[:, dc, :],
                                             start=(dc == 0), stop=False)
                        nc.tensor.matmul(y1_ps,
                                         w1ac_sb[:, hd * 2:hd * 2 + 2, osl],
                                         hT8[:, hd * 2:hd * 2 + 2, ssl],
                                         start=False, stop=False, perf_mode=DR)
                        nc.tensor.matmul(y1_ps, w1d_sb[:, :, osl], attnT,
                                         start=False, stop=True, perf_mode=DR)
                        nc.scalar.activation(y1T[:, ot, :], y1_ps, AF.Silu,
                                             scale=1.0 / 512.0)
                    for ot in range(2):
                        osl = slice(ot * 128, (ot + 1) * 128)
                        gin_ps = ps2.tile([128, 512], f32,
                                          name=f"ginp{sw}_{hd}_{ot}",
                                          tag="mm", bufs=3)
                        for dc in range(2):
                            nc.tensor.matmul(gin_ps, w2_sb[:, dc, osl],
                                             y1T[:, dc, :],
                                             start=(dc == 0), stop=(dc == 1))
                        nc.scalar.activation(ginT[:, hd * 2 + ot, :], gin_ps,
                                             AF.Copy)

                # down-projection for this window's four s-tiles
                for r in range(4):
                    st = sw * 4 + r
                    stsl = slice(st * 128, (st + 1) * 128)
                    rsl = slice(r * 128, (r + 1) * 128)
                    o_sb = outp.tile([128, HID], f32, name=f"o{st}", tag="o")
                    for nw in range(2):
                        d_ps = ps2.tile([128, 512], f32, name=f"d{st}_{nw}",
                                        tag="mm", bufs=3)
                        for j in range(LOC // 128):
                            nc.tensor.matmul(d_ps, ginT[:, j, rsl],
                                             wd_sb[:, j, nw * 512:(nw + 1) * 512],
                                             start=(j == 0),
                                             stop=(j == LOC // 128 - 1))
                        if nw == 0:
                            nc.vector.tensor_copy(o_sb[:, nw * 512:(nw + 1) * 512],
                                                  d_ps)
                        else:
                            nc.scalar.activation(o_sb[:, nw * 512:(nw + 1) * 512],
                                                 d_ps, AF.Copy)
                    nc.sync.dma_start(out_d[stsl, :], o_sb)

    nc.compile()
    return nc


def _prep_in_maps(x, adjacency, Wg, Wu, Wd, eps, alpha, Wq, Wk, W1, W2):
    f = lambda a: np.ascontiguousarray(a, dtype=np.float32)
    x, adjacency = f(x), f(adjacency)
    Wg, Wu, Wd, Wq, Wk, W1, W2 = map(f, (Wg, Wu, Wd, Wq, Wk, W1, W2))
    eps, alpha = f(eps), f(alpha)
    b16 = lambda a: np.ascontiguousarray(a).astype(BF16)
    f8 = lambda a: np.ascontiguousarray(a).astype(FP8)

    NST, NKC, NSW, NTC = S // 128, HID // 128, S // 512, S // 128
    # x blocks: [st, p, c, sl] = x[0, st*128+sl, c*128+p]
    xb = b16(x[0].reshape(NST, 128, NKC, 128).transpose(0, 3, 2, 1))

    W1a = W1[:, 0:D]
    W1b = W1[:, D:2 * D]
    W1c = W1[:, 2 * D:3 * D]
    W1d = W1[:, 3 * D:4 * D]

    in_maps = []
    for i in range(NCORES):
        hs = slice(i * HPC, (i + 1) * HPC)
        c0, c1 = i * LOC, (i + 1) * LOC
        # adj blocks: [sw, hd, p, tt, sl] = 16*adj[hd, sw*512+sl, tt*128+p]
        a = adjacency[0, hs].reshape(HPC, NSW, 512, NTC, 128)
        adjb = b16(16.0 * a.transpose(1, 0, 4, 3, 2))
        w1ac = np.stack([64.0 * ((1.0 + eps[h]) * W1a + W1c).T
                         for h in range(i * HPC, (i + 1) * HPC)])
        w1b = np.stack([32.0 * alpha[h] * W1b.T
                        for h in range(i * HPC, (i + 1) * HPC)])
        in_maps.append({
            "xb": xb,
            "wgb": b16(Wg[c0:c1].T.reshape(NKC, 128, LOC)),
            "wub": b16(Wu[c0:c1].T.reshape(NKC, 128, LOC)),
            "adjb": adjb,
            "wqT": f8(Wq[hs].transpose(0, 2, 1) * 8.0),
            "wkT": f8(Wk[hs].transpose(0, 2, 1) * 8.0),
            "w1acT": f8(w1ac),
            "w1bT": b16(w1b),
            "w1dT": f8(64.0 * W1d.T),
            "w2T": b16(W2.T),
            "wdT": b16(Wd[:, c0:c1].T),
        })
    return in_maps


def _run(inputs, trace=False, trace_kwargs=None):
    from concourse.bass_utils import run_bass_kernel_spmd

    if "nc" not in _CACHE:
        _CACHE["nc"] = _build_nc()
    nc = _CACHE["nc"]
    in_maps = _prep_in_maps(**inputs)
    res = run_bass_kernel_spmd(nc, in_maps, list(range(NCORES)),
                               trace=trace, **(trace_kwargs or {}))
    out = np.zeros((S, HID), np.float32)
    for r in res.results:
        out += r["out"]
    return out.reshape(B, S, HID), res


def kernel(**inputs) -> np.ndarray:
    out, _ = _run(inputs, trace=False)
    return out


# revision 7
# speedup vs baseline: 1.2523x; 1.1052x over previous
"""Trainium2 Bass kernel for nn_LlamaMLP_HalfwayGIN_MultiAggregration.

Sharding: 16 heads -> 8 cores (2 heads/core). Each core computes its two
heads' full pipeline plus the partial down-projection; host sums partials.

v2: fp8 DoubleRow for the attention-aggregation + denominator matmuls
(t-pairs, contraction 256), fp8 DR for the small y1 terms, contiguous
host-side DMA layouts (one 2MB DMA per adjacency stripe), sw-outer loop
with the GIN MLP + down-projection interleaved per s-window.

Scale folding (host):
  hT8/h8 = 8*h (fp8); wq8 = 8*Wq^T, wk8 = 8*Wk^T  -> scores psum = 4096*QK^T
  exp scale = 1/(4096*sqrt(D)) = 1/65536
  adj16 = 16*adjacency^T (bf16)  -> em8 = 16*em, sumT = 16*sum_raw
  w1b = 32*alpha*W1b^T (bf16: 512/16), w1ac8 = 64*((1+eps)W1a+W1c)^T (fp8:
  512/8 vs h8), w1d8 = 64*W1d^T (fp8: 512/8 vs attnT=8*attn), silu scale 1/512
"""

import math
import os
import numpy as np
import ml_dtypes

B, S, HID, NH, INTER = 1, 2048, 1024, 16, 4096
D = 256
NCORES = 8
HPC = NH // NCORES          # 2 heads per core
LOC = HPC * D               # 512 local intermediate dims
BF16 = ml_dtypes.bfloat16
FP8 = ml_dtypes.float8_e4m3

_CACHE = {}


def _build_nc():
    import concourse.mybir as mybir
    import concourse.tile as tile
    from concourse import bacc
    from concourse.masks import make_identity
    from contextlib import ExitStack

    f32 = mybir.dt.float32
    bf16 = mybir.dt.bfloat16
    fp8 = mybir.dt.float8e4
    AF = mybir.ActivationFunctionType
    DR = mybir.MatmulPerfMode.DoubleRow

    nc = bacc.Bacc("TRN2", target_bir_lowering=False, debug=False)

    NST = S // 128            # 16 s-tiles
    NSW = S // 512            # 4 s-windows
    NTC = S // 128            # 16 t-chunks
    NPR = NTC // 2            # 8 t-pairs
    NKC = HID // 128          # 8 k-chunks

    x_d = nc.dram_tensor("xb", [NST, 128, NKC, 128], bf16, kind="ExternalInput")
    wg_d = nc.dram_tensor("wgb", [NKC, 128, LOC], bf16, kind="ExternalInput")
    wu_d = nc.dram_tensor("wub", [NKC, 128, LOC], bf16, kind="ExternalInput")
    adj_d = nc.dram_tensor("adjb", [NSW, HPC, 128, NTC, 512], bf16,
                           kind="ExternalInput")
    wq_d = nc.dram_tensor("wqT", [HPC, D, D], fp8, kind="ExternalInput")
    wk_d = nc.dram_tensor("wkT", [HPC, D, D], fp8, kind="ExternalInput")
    w1ac_d = nc.dram_tensor("w1acT", [HPC, D, D], fp8, kind="ExternalInput")
    w1b_d = nc.dram_tensor("w1bT", [HPC, D, D], bf16, kind="ExternalInput")
    w1d_d = nc.dram_tensor("w1dT", [D, D], fp8, kind="ExternalInput")
    w2_d = nc.dram_tensor("w2T", [D, D], bf16, kind="ExternalInput")
    wd_d = nc.dram_tensor("wdT", [LOC, HID], bf16, kind="ExternalInput")
    out_d = nc.dram_tensor("out", [S, HID], f32, kind="ExternalOutput")

    with ExitStack() as es:
        tc = es.enter_context(tile.TileContext(nc))

        persist = es.enter_context(tc.tile_pool(name="persist", bufs=1))
        h_all = persist.tile([128, NST, LOC], bf16, name="h_all")
        h8_all = persist.tile([128, NST, LOC], fp8, name="h8_all")
        hT8 = persist.tile([128, 2 * HPC, S], fp8, name="hT8")
        qT = persist.tile([128, HPC, 2, S], fp8, name="qT")
        kT = persist.tile([128, HPC, 2, S], fp8, name="kT")

        wpool = es.enter_context(tc.tile_pool(name="weights", bufs=1))
        wq_sb = wpool.tile([128, 2 * HPC, D], fp8, name="wq_sb")
        wk_sb = wpool.tile([128, 2 * HPC, D], fp8, name="wk_sb")
        w1ac_sb = wpool.tile([128, 2 * HPC, D], fp8, name="w1ac_sb")
        w1b_sb = wpool.tile([128, 2 * HPC, D], bf16, name="w1b_sb")
        w1d_sb = wpool.tile([128, 2, D], fp8, name="w1d_sb")
        w2_sb = wpool.tile([128, 2, D], bf16, name="w2_sb")
        wd_sb = wpool.tile([128, LOC // 128, HID], bf16, name="wd_sb")

        misc = es.enter_context(tc.tile_pool(name="misc", bufs=1))
        id_sb = misc.tile([128, 128], bf16, name="id_sb")
        ones2 = misc.tile([128, 2, 128], fp8, name="ones2")

        make_identity(nc, id_sb)
        nc.vector.memset(ones2, 1.0)

        adjpool = es.enter_context(tc.tile_pool(name="adj", bufs=1))

        # ---- phase 1: h = silu(x@WgT)*(x@WuT); hT8, h8 side copies ----
        with tc.tile_pool(name="xpool", bufs=1) as xpool, \
             tc.tile_pool(name="ps1", bufs=1, space="PSUM") as ps1, \
             tc.tile_pool(name="hstage", bufs=3) as hstage:
            x_sb = xpool.tile([128, NST, NKC, 128], bf16, name="x_sb")
            wg_sb = xpool.tile([128, NKC, LOC], bf16, name="wg_sb")
            wu_sb = xpool.tile([128, NKC, LOC], bf16, name="wu_sb")
            # wg/wu + first-half x interleaved on sync; rest of x on
            # gpsimd; small weights and adj stripes queue behind on sync so
            # phase-1 loads get the HBM bandwidth first
            for c in range(NKC):
                nc.sync.dma_start(wg_sb[:, c, :], wg_d[c])
                nc.sync.dma_start(wu_sb[:, c, :], wu_d[c])
                nc.sync.dma_start(x_sb[:, c], x_d[c])
            for st in range(NKC, NST):
                nc.gpsimd.dma_start(x_sb[:, st], x_d[st])
            nc.sync.dma_start(wq_sb, wq_d.rearrange("h (c p) e -> p (h c) e", p=128))
            nc.sync.dma_start(wk_sb, wk_d.rearrange("h (c p) e -> p (h c) e", p=128))
            nc.sync.dma_start(w1ac_sb, w1ac_d.rearrange("h (c p) o -> p (h c) o", p=128))
            nc.sync.dma_start(w1b_sb, w1b_d.rearrange("h (c p) o -> p (h c) o", p=128))
            nc.sync.dma_start(w1d_sb, w1d_d.rearrange("(c p) o -> p c o", p=128))
            nc.sync.dma_start(w2_sb, w2_d.rearrange("(c p) o -> p c o", p=128))
            nc.sync.dma_start(wd_sb, wd_d.rearrange("(c p) o -> p c o", p=128))
            adj_tiles = {}
            for sw in range(NSW):
                for hd in range(HPC):
                    a = adjpool.tile([128, NTC, 512], bf16,
                                     name=f"adj{sw}_{hd}", tag="adj", bufs=3)
                    nc.sync.dma_start(a, adj_d[sw, hd])
                    adj_tiles[(sw, hd)] = a

            def do_tr(st):
                # transpose s-tile st's four d-chunks (pipelined one behind)
                tr_ps = ps1.tile([128, 2 * HPC, 128], bf16, name=f"tr{st}",
                                 tag="tr", bufs=2)
                for j in range(2 * HPC):
                    col0 = j * 128
                    nc.tensor.transpose(tr_ps[:, j, :],
                                        h_all[:, st, col0:col0 + 128], id_sb)
                stsl = slice(st * 128, (st + 1) * 128)
                nc.scalar.activation(hT8[:, :, stsl], tr_ps, AF.Copy, scale=8.0)

            for st in range(NST):
                g_ps = ps1.tile([128, LOC], f32, name=f"g{st}", tag="g", bufs=2)
                u_ps = ps1.tile([128, LOC], f32, name=f"u{st}", tag="u", bufs=2)
                for c in range(NKC):
                    lhsT = x_sb[:, st, c, :]
                    nc.tensor.matmul(g_ps, lhsT, wg_sb[:, c, :],
                                     start=(c == 0), stop=(c == NKC - 1))
                    nc.tensor.matmul(u_ps, lhsT, wu_sb[:, c, :],
                                     start=(c == 0), stop=(c == NKC - 1))
                if st >= 1:
                    do_tr(st - 1)
                sg = hstage.tile([128, LOC], bf16, name=f"sg{st}", tag="sg")
                nc.scalar.activation(sg, g_ps, AF.Silu)
                nc.vector.tensor_mul(h_all[:, st, :], sg, u_ps)
                nc.scalar.activation(h8_all[:, st, :], h_all[:, st, :],
                                     AF.Copy, scale=8.0)
            do_tr(NST - 1)

            # QK projections for both heads (fp8 DoubleRow, contraction 256)
            for hd in range(HPC):
                for w_sb, dstT in ((wq_sb, qT), (wk_sb, kT)):
                    for et in range(2):
                        for sw in range(NSW):
                            ssl = slice(sw * 512, (sw + 1) * 512)
                            ps = ps1.tile([128, 512], f32,
                                          name=f"qk{hd}_{et}_{sw}", tag="g",
                                          bufs=2)
                            nc.tensor.matmul(
                                ps,
                                w_sb[:, hd * 2:hd * 2 + 2, et * 128:(et + 1) * 128],
                                hT8[:, hd * 2:hd * 2 + 2, ssl],
                                start=True, stop=True, perf_mode=DR)
                            nc.scalar.activation(dstT[:, hd, et, ssl], ps,
                                                 AF.Copy)

        # ---- phase 2+3 fused, sw-outer; p3 of window sw-1 interleaved
        # into window sw's attention pair loop ----
        with tc.tile_pool(name="stream", bufs=1) as strm, \
             tc.tile_pool(name="outp", bufs=2) as outp, \
             tc.tile_pool(name="ps2", bufs=1, space="PSUM") as ps2:

            def make_p3(sw, hd_res):
                """Phase-3 chunk closures for window sw (16 chunks)."""
                ssl = slice(sw * 512, (sw + 1) * 512)
                ginT = strm.tile([128, 2 * HPC, 512], bf16, name=f"gin{sw}",
                                 tag="gin", bufs=2)
                y1Ts = [strm.tile([128, 2, 512], bf16, name=f"y1{sw}_{hd}",
                                  tag=f"y1_{hd}", bufs=2) for hd in range(HPC)]
                chunks = []

                def y1_chunk(hd, ot):
                    sumT, attnT = hd_res[hd]
                    osl = slice(ot * 128, (ot + 1) * 128)
                    y1_ps = ps2.tile([128, 512], f32,
                                     name=f"y1p{sw}_{hd}_{ot}", tag="mm",
                                     bufs=3)
                    for dc in range(2):
                        nc.tensor.matmul(y1_ps, w1b_sb[:, hd * 2 + dc, osl],
                                         sumT[:, dc, :],
                                         start=(dc == 0), stop=False)
                    nc.tensor.matmul(y1_ps, w1ac_sb[:, hd * 2:hd * 2 + 2, osl],
                                     hT8[:, hd * 2:hd * 2 + 2, ssl],
                                     start=False, stop=False, perf_mode=DR)
                    nc.tensor.matmul(y1_ps, w1d_sb[:, :, osl], attnT,
                                     start=False, stop=True, perf_mode=DR)
                    nc.scalar.activation(y1Ts[hd][:, ot, :], y1_ps, AF.Silu,
                                         scale=1.0 / 512.0)

                def gin_chunk(hd, ot):
                    osl = slice(ot * 128, (ot + 1) * 128)
                    gin_ps = ps2.tile([128, 512], f32,
                                      name=f"ginp{sw}_{hd}_{ot}", tag="mm",
                                      bufs=3)
                    for dc in range(2):
                        nc.tensor.matmul(gin_ps, w2_sb[:, dc, osl],
                                         y1Ts[hd][:, dc, :],
                                         start=(dc == 0), stop=(dc == 1))
                    nc.scalar.activation(ginT[:, hd * 2 + ot, :], gin_ps,
                                         AF.Copy)

                o_sbs = {}

                def down_chunk(r, nw):
                    st = sw * 4 + r
                    rsl = slice(r * 128, (r + 1) * 128)
                    if nw == 0:
                        o_sbs[r] = outp.tile([128, HID], f32, name=f"o{st}",
                                             tag="o")
                    o_sb = o_sbs[r]
                    d_ps = ps2.tile([128, 512], f32, name=f"d{st}_{nw}",
                                    tag="mm", bufs=3)
                    for j in range(LOC // 128):
                        nc.tensor.matmul(d_ps, ginT[:, j, rsl],
                                         wd_sb[:, j, nw * 512:(nw + 1) * 512],
                                         start=(j == 0),
                                         stop=(j == LOC // 128 - 1))
                    if nw == 0:
                        nc.vector.tensor_copy(o_sb[:, 0:512], d_ps)
                    else:
                        nc.scalar.activation(o_sb[:, 512:1024], d_ps, AF.Copy)
                        stsl = slice(st * 128, (st + 1) * 128)
                        nc.gpsimd.dma_start(out_d[stsl, :], o_sb)

                for hd in range(HPC):
                    for ot in range(2):
                        chunks.append(lambda hd=hd, ot=ot: y1_chunk(hd, ot))
                for hd in range(HPC):
                    for ot in range(2):
                        chunks.append(lambda hd=hd, ot=ot: gin_chunk(hd, ot))
                for r in range(4):
                    for nw in range(2):
                        chunks.append(lambda r=r, nw=nw: down_chunk(r, nw))
                return chunks

            pending = []
            for sw in range(NSW):
                ssl = slice(sw * 512, (sw + 1) * 512)
                hd_res = []
                for hd in range(HPC):
                    adj_sb = adj_tiles[(sw, hd)]
                    sum_ps = ps2.tile([128, 2, 512], f32,
                                      name=f"sum{sw}_{hd}", tag="sum")
                    att_ps = ps2.tile([128, 2, 512], f32,
                                      name=f"att{sw}_{hd}", tag="att")
                    den_ps = ps2.tile([128, 512], f32,
                                      name=f"den{sw}_{hd}", tag="den", bufs=1)
                    em_tiles = {}
                    for pr in range(NPR + 1):
                        if pr < NPR:
                            em8 = strm.tile([128, 2, 512], fp8,
                                            name=f"em{sw}_{hd}_{pr}",
                                            tag="em", bufs=4)
                            ex = strm.tile([128, 2, 512], bf16,
                                           name=f"ex{sw}_{hd}_{pr}",
                                           tag="ex", bufs=3)
                            for i in range(2):
                                t = 2 * pr + i
                                tsl = slice(t * 128, (t + 1) * 128)
                                sc_ps = ps2.tile([128, 512], f32,
                                                 name=f"sc{sw}_{hd}_{pr}_{i}",
                                                 tag="mm", bufs=3)
                                nc.tensor.matmul(sc_ps, kT[:, hd, :, tsl],
                                                 qT[:, hd, :, ssl],
                                                 start=True, stop=True,
                                                 perf_mode=DR)
                                nc.scalar.activation(ex[:, i, :], sc_ps, AF.Exp,
                                                     scale=1.0 / 65536.0)
                            nc.vector.tensor_mul(em8, ex,
                                                 adj_sb[:, 2 * pr:2 * pr + 2, :])
                            em_tiles[pr] = em8
                        if pr >= 1:
                            p = pr - 1
                            em_p = em_tiles.pop(p)
                            first, last = p == 0, p == NPR - 1
                            for dc in range(2):
                                c0 = hd * D + dc * 128
                                for i in range(2):
                                    t = 2 * p + i
                                    nc.tensor.matmul(
                                        sum_ps[:, dc, :],
                                        h_all[:, t, c0:c0 + 128],
                                        adj_sb[:, t, :],
                                        start=(first and i == 0),
                                        stop=(last and i == 1))
                            nc.tensor.matmul(den_ps, ones2, em_p,
                                             start=first, stop=last,
                                             perf_mode=DR)
                            for dc in range(2):
                                c0 = hd * D + dc * 128
                                nc.tensor.matmul(
                                    att_ps[:, dc, :],
                                    h8_all[:, 2 * p:2 * p + 2, c0:c0 + 128],
                                    em_p, start=first, stop=last,
                                    perf_mode=DR)
                            if pending:
                                pending.pop(0)()

                    # evictions: sumT copy, recip (replicated den -> direct
                    # multiplier), attnT muls; all on vector
                    sumT = strm.tile([128, 2, 512], bf16, name=f"sumT{sw}_{hd}",
                                     tag="sumT", bufs=2)
                    nc.vector.tensor_copy(sumT, sum_ps)
                    rb_sb = strm.tile([128, 512], f32, name=f"rb{sw}_{hd}",
                                      tag="rb", bufs=2)
                    nc.vector.reciprocal_approx_fast(rb_sb, den_ps)
                    attnT = strm.tile([128, 2, 512], fp8, name=f"at{sw}_{hd}",
                                      tag="attnT", bufs=2)
                    for dc in range(2):
                        nc.vector.tensor_mul(attnT[:, dc, :],
                                             att_ps[:, dc, :], rb_sb)
                    hd_res.append((sumT, attnT))
                pending.extend(make_p3(sw, hd_res))
            for c in pending:
                c()

    nc.compile()
    return nc


def _prep_in_maps(x, adjacency, Wg, Wu, Wd, eps, alpha, Wq, Wk, W1, W2):
    f = lambda a: np.ascontiguousarray(a, dtype=np.float32)
    x, adjacency = f(x), f(adjacency)
    Wg, Wu, Wd, Wq, Wk, W1, W2 = map(f, (Wg, Wu, Wd, Wq, Wk, W1, W2))
    eps, alpha = f(eps), f(alpha)
    b16 = lambda a: np.ascontiguousarray(a).astype(BF16)
    f8 = lambda a: np.ascontiguousarray(a).astype(FP8)

    NST, NKC, NSW, NTC = S // 128, HID // 128, S // 512, S // 128
    # x blocks: [st, p, c, sl] = x[0, st*128+sl, c*128+p]
    xb = b16(x[0].reshape(NST, 128, NKC, 128).transpose(0, 3, 2, 1))

    W1a = W1[:, 0:D]
    W1b = W1[:, D:2 * D]
    W1c = W1[:, 2 * D:3 * D]
    W1d = W1[:, 3 * D:4 * D]

    in_maps = []
    for i in range(NCORES):
        hs = slice(i * HPC, (i + 1) * HPC)
        c0, c1 = i * LOC, (i + 1) * LOC
        # adj blocks: [sw, hd, p, tt, sl] = 16*adj[hd, sw*512+sl, tt*128+p]
        a = adjacency[0, hs].reshape(HPC, NSW, 512, NTC, 128)
        adjb = b16(16.0 * a.transpose(1, 0, 4, 3, 2))
        w1ac = np.stack([64.0 * ((1.0 + eps[h]) * W1a + W1c).T
                         for h in range(i * HPC, (i + 1) * HPC)])
        w1b = np.stack([32.0 * alpha[h] * W1b.T
                        for h in range(i * HPC, (i + 1) * HPC)])
        in_maps.append({
            "xb": xb,
            "wgb": b16(Wg[c0:c1].T.reshape(NKC, 128, LOC)),
            "wub": b16(Wu[c0:c1].T.reshape(NKC, 128, LOC)),
            "adjb": adjb,
            "wqT": f8(Wq[hs].transpose(0, 2, 1) * 8.0),
            "wkT": f8(Wk[hs].transpose(0, 2, 1) * 8.0),
            "w1acT": f8(w1ac),
            "w1bT": b16(w1b),
            "w1dT": f8(64.0 * W1d.T),
            "w2T": b16(W2.T),
            "wdT": b16(Wd[:, c0:c1].T),
        })
    return in_maps


def _run(inputs, trace=False, trace_kwargs=None):
    from concourse.bass_utils import run_bass_kernel_spmd

    if "nc" not in _CACHE:
        _CACHE["nc"] = _build_nc()
    nc = _CACHE["nc"]
    in_maps = _prep_in_maps(**inputs)
    res = run_bass_kernel_spmd(nc, in_maps, list(range(NCORES)),
                               trace=trace, **(trace_kwargs or {}))
    out = np.zeros((S, HID), np.float32)
    for r in res.results:
        out += r["out"]
    return out.reshape(B, S, HID), res


def kernel(**inputs) -> np.ndarray:
    out, _ = _run(inputs, trace=False)
    return out


# revision 8
# speedup vs baseline: 1.2991x; 1.0374x over previous
"""Trainium2 Bass kernel for nn_LlamaMLP_HalfwayGIN_MultiAggregration.

Sharding: 16 heads -> 8 cores (2 heads/core). Each core computes its two
heads' full pipeline plus the partial down-projection; host sums partials.

v2: fp8 DoubleRow for the attention-aggregation + denominator matmuls
(t-pairs, contraction 256), fp8 DR for the small y1 terms, contiguous
host-side DMA layouts (one 2MB DMA per adjacency stripe), sw-outer loop
with the GIN MLP + down-projection interleaved per s-window.

Scale folding (host):
  hT8/h8 = 8*h (fp8); wq8 = 8*Wq^T, wk8 = 8*Wk^T  -> scores psum = 4096*QK^T
  exp scale = 1/(4096*sqrt(D)) = 1/65536
  adj16 = 16*adjacency^T (bf16)  -> em8 = 16*em, sumT = 16*sum_raw
  w1b = 32*alpha*W1b^T (bf16: 512/16), w1ac8 = 64*((1+eps)W1a+W1c)^T (fp8:
  512/8 vs h8), w1d8 = 64*W1d^T (fp8: 512/8 vs attnT=8*attn), silu scale 1/512
"""

import math
import os
import numpy as np
import ml_dtypes

B, S, HID, NH, INTER = 1, 2048, 1024, 16, 4096
D = 256
NCORES = 8
HPC = NH // NCORES          # 2 heads per core
LOC = HPC * D               # 512 local intermediate dims
BF16 = ml_dtypes.bfloat16
FP8 = ml_dtypes.float8_e4m3

_CACHE = {}


def _build_nc():
    import concourse.mybir as mybir
    import concourse.tile as tile
    from concourse import bacc
    from concourse.masks import make_identity
    from contextlib import ExitStack

    f32 = mybir.dt.float32
    bf16 = mybir.dt.bfloat16
    fp8 = mybir.dt.float8e4
    AF = mybir.ActivationFunctionType
    DR = mybir.MatmulPerfMode.DoubleRow

    nc = bacc.Bacc("TRN2", target_bir_lowering=False, debug=False)

    NST = S // 128            # 16 s-tiles
    NSW = S // 512            # 4 s-windows
    NTC = S // 128            # 16 t-chunks
    NPR = NTC // 2            # 8 t-pairs
    NKC = HID // 128          # 8 k-chunks

    x_d = nc.dram_tensor("xb", [NST, 128, NKC, 128], bf16, kind="ExternalInput")
    wg_d = nc.dram_tensor("wgb", [NKC, 128, LOC], bf16, kind="ExternalInput")
    wu_d = nc.dram_tensor("wub", [NKC, 128, LOC], bf16, kind="ExternalInput")
    adj_d = nc.dram_tensor("adjb", [NSW, HPC, 128, NTC, 512], bf16,
                           kind="ExternalInput")
    wq_d = nc.dram_tensor("wqT", [HPC, D, D], fp8, kind="ExternalInput")
    wk_d = nc.dram_tensor("wkT", [HPC, D, D], fp8, kind="ExternalInput")
    w1ac_d = nc.dram_tensor("w1acT", [HPC, D, D], fp8, kind="ExternalInput")
    w1b_d = nc.dram_tensor("w1bT", [HPC, D, D], bf16, kind="ExternalInput")
    w1d_d = nc.dram_tensor("w1dT", [D, D], fp8, kind="ExternalInput")
    w2_d = nc.dram_tensor("w2T", [D, D], bf16, kind="ExternalInput")
    wd_d = nc.dram_tensor("wdT", [LOC, HID], bf16, kind="ExternalInput")
    out_d = nc.dram_tensor("out", [S, HID], f32, kind="ExternalOutput")

    with ExitStack() as es:
        tc = es.enter_context(tile.TileContext(nc))

        persist = es.enter_context(tc.tile_pool(name="persist", bufs=1))
        h_all = persist.tile([128, NST, LOC], bf16, name="h_all")
        h8_all = persist.tile([128, NST, LOC], fp8, name="h8_all")
        hT8 = persist.tile([128, 2 * HPC, S], fp8, name="hT8")
        qT = persist.tile([128, HPC, 2, S], fp8, name="qT")
        kT = persist.tile([128, HPC, 2, S], fp8, name="kT")

        wpool = es.enter_context(tc.tile_pool(name="weights", bufs=1))
        wq_sb = wpool.tile([128, 2 * HPC, D], fp8, name="wq_sb")
        wk_sb = wpool.tile([128, 2 * HPC, D], fp8, name="wk_sb")
        w1ac_sb = wpool.tile([128, 2 * HPC, D], fp8, name="w1ac_sb")
        w1b_sb = wpool.tile([128, 2 * HPC, D], bf16, name="w1b_sb")
        w1d_sb = wpool.tile([128, 2, D], fp8, name="w1d_sb")
        w2_sb = wpool.tile([128, 2, D], bf16, name="w2_sb")
        wd_sb = wpool.tile([128, LOC // 128, HID], bf16, name="wd_sb")

        misc = es.enter_context(tc.tile_pool(name="misc", bufs=1))
        id_sb = misc.tile([128, 128], bf16, name="id_sb")
        ones2 = misc.tile([128, 2, 128], fp8, name="ones2")

        make_identity(nc, id_sb)
        nc.vector.memset(ones2, 1.0)

        adjpool = es.enter_context(tc.tile_pool(name="adj", bufs=1))

        # ---- phase 1: h = silu(x@WgT)*(x@WuT); hT8, h8 side copies ----
        with tc.tile_pool(name="xpool", bufs=1) as xpool, \
             tc.tile_pool(name="ps1", bufs=1, space="PSUM") as ps1, \
             tc.tile_pool(name="hstage", bufs=3) as hstage:
            x_sb = xpool.tile([128, NST, NKC, 128], bf16, name="x_sb")
            wg_sb = xpool.tile([128, NKC, LOC], bf16, name="wg_sb")
            wu_sb = xpool.tile([128, NKC, LOC], bf16, name="wu_sb")
            # wg/wu + first-half x interleaved on sync; rest of x on
            # gpsimd; small weights and adj stripes queue behind on sync so
            # phase-1 loads get the HBM bandwidth first
            for c in range(NKC):
                nc.sync.dma_start(x_sb[:, c], x_d[c])
                nc.sync.dma_start(wg_sb[:, c, :], wg_d[c])
                nc.sync.dma_start(wu_sb[:, c, :], wu_d[c])
            for st in range(NKC, NST):
                nc.gpsimd.dma_start(x_sb[:, st], x_d[st])
            nc.sync.dma_start(wq_sb, wq_d.rearrange("h (c p) e -> p (h c) e", p=128))
            nc.sync.dma_start(wk_sb, wk_d.rearrange("h (c p) e -> p (h c) e", p=128))
            nc.sync.dma_start(w1ac_sb, w1ac_d.rearrange("h (c p) o -> p (h c) o", p=128))
            nc.sync.dma_start(w1b_sb, w1b_d.rearrange("h (c p) o -> p (h c) o", p=128))
            nc.sync.dma_start(w1d_sb, w1d_d.rearrange("(c p) o -> p c o", p=128))
            nc.sync.dma_start(w2_sb, w2_d.rearrange("(c p) o -> p c o", p=128))
            nc.sync.dma_start(wd_sb, wd_d.rearrange("(c p) o -> p c o", p=128))
            adj_tiles = {}
            for sw in range(NSW):
                for hd in range(HPC):
                    a = adjpool.tile([128, NTC, 512], bf16,
                                     name=f"adj{sw}_{hd}", tag="adj", bufs=3)
                    nc.sync.dma_start(a, adj_d[sw, hd])
                    adj_tiles[(sw, hd)] = a

            def do_tr(st):
                # transpose s-tile st's four d-chunks (pipelined one behind)
                tr_ps = ps1.tile([128, 2 * HPC, 128], bf16, name=f"tr{st}",
                                 tag="tr", bufs=2)
                for j in range(2 * HPC):
                    col0 = j * 128
                    nc.tensor.transpose(tr_ps[:, j, :],
                                        h_all[:, st, col0:col0 + 128], id_sb)
                stsl = slice(st * 128, (st + 1) * 128)
                nc.vector.tensor_scalar_mul(hT8[:, :, stsl], tr_ps, 8.0)

            for st in range(NST):
                g_ps = ps1.tile([128, LOC], f32, name=f"g{st}", tag="g", bufs=2)
                u_ps = ps1.tile([128, LOC], f32, name=f"u{st}", tag="u", bufs=2)
                for c in range(NKC):
                    lhsT = x_sb[:, st, c, :]
                    nc.tensor.matmul(g_ps, lhsT, wg_sb[:, c, :],
                                     start=(c == 0), stop=(c == NKC - 1))
                    nc.tensor.matmul(u_ps, lhsT, wu_sb[:, c, :],
                                     start=(c == 0), stop=(c == NKC - 1))
                if st >= 1:
                    do_tr(st - 1)
                sg = hstage.tile([128, LOC], bf16, name=f"sg{st}", tag="sg")
                nc.scalar.activation(sg, g_ps, AF.Silu)
                nc.vector.tensor_mul(h_all[:, st, :], sg, u_ps)
                nc.vector.tensor_scalar_mul(h8_all[:, st, :],
                                            h_all[:, st, :], 8.0)
            do_tr(NST - 1)

            # QK projections for both heads (fp8 DoubleRow, contraction 256)
            for hd in range(HPC):
                for w_sb, dstT in ((wq_sb, qT), (wk_sb, kT)):
                    for et in range(2):
                        for sw in range(NSW):
                            ssl = slice(sw * 512, (sw + 1) * 512)
                            ps = ps1.tile([128, 512], f32,
                                          name=f"qk{hd}_{et}_{sw}", tag="g",
                                          bufs=2)
                            nc.tensor.matmul(
                                ps,
                                w_sb[:, hd * 2:hd * 2 + 2, et * 128:(et + 1) * 128],
                                hT8[:, hd * 2:hd * 2 + 2, ssl],
                                start=True, stop=True, perf_mode=DR)
                            nc.vector.tensor_copy(dstT[:, hd, et, ssl], ps)

        # ---- phase 2+3 fused, sw-outer; p3 of window sw-1 interleaved
        # into window sw's attention pair loop ----
        with tc.tile_pool(name="stream", bufs=1) as strm, \
             tc.tile_pool(name="outp", bufs=2) as outp, \
             tc.tile_pool(name="ps2", bufs=1, space="PSUM") as ps2:

            def make_p3(sw, hd_res):
                """Phase-3 chunk closures for window sw (16 chunks)."""
                ssl = slice(sw * 512, (sw + 1) * 512)
                ginT = strm.tile([128, 2 * HPC, 512], bf16, name=f"gin{sw}",
                                 tag="gin", bufs=2)
                y1Ts = [strm.tile([128, 2, 512], bf16, name=f"y1{sw}_{hd}",
                                  tag=f"y1_{hd}", bufs=2) for hd in range(HPC)]
                chunks = []

                def y1_chunk(hd, ot):
                    sumT, attnT = hd_res[hd]
                    osl = slice(ot * 128, (ot + 1) * 128)
                    y1_ps = ps2.tile([128, 512], f32,
                                     name=f"y1p{sw}_{hd}_{ot}", tag="mm",
                                     bufs=3)
                    for dc in range(2):
                        nc.tensor.matmul(y1_ps, w1b_sb[:, hd * 2 + dc, osl],
                                         sumT[:, dc, :],
                                         start=(dc == 0), stop=False)
                    nc.tensor.matmul(y1_ps, w1ac_sb[:, hd * 2:hd * 2 + 2, osl],
                                     hT8[:, hd * 2:hd * 2 + 2, ssl],
                                     start=False, stop=False, perf_mode=DR)
                    nc.tensor.matmul(y1_ps, w1d_sb[:, :, osl], attnT,
                                     start=False, stop=True, perf_mode=DR)
                    nc.scalar.activation(y1Ts[hd][:, ot, :], y1_ps, AF.Silu,
                                         scale=1.0 / 512.0)

                def gin_chunk(hd, ot):
                    osl = slice(ot * 128, (ot + 1) * 128)
                    gin_ps = ps2.tile([128, 512], f32,
                                      name=f"ginp{sw}_{hd}_{ot}", tag="mm",
                                      bufs=3)
                    for dc in range(2):
                        nc.tensor.matmul(gin_ps, w2_sb[:, dc, osl],
                                         y1Ts[hd][:, dc, :],
                                         start=(dc == 0), stop=(dc == 1))
                    nc.scalar.activation(ginT[:, hd * 2 + ot, :], gin_ps,
                                         AF.Copy)

                o_sbs = {}

                def down_chunk(r, nw):
                    st = sw * 4 + r
                    rsl = slice(r * 128, (r + 1) * 128)
                    if nw == 0:
                        o_sbs[r] = outp.tile([128, HID], f32, name=f"o{st}",
                                             tag="o")
                    o_sb = o_sbs[r]
                    d_ps = ps2.tile([128, 512], f32, name=f"d{st}_{nw}",
                                    tag="mm", bufs=3)
                    for j in range(LOC // 128):
                        nc.tensor.matmul(d_ps, ginT[:, j, rsl],
                                         wd_sb[:, j, nw * 512:(nw + 1) * 512],
                                         start=(j == 0),
                                         stop=(j == LOC // 128 - 1))
                    if nw == 0:
                        nc.vector.tensor_copy(o_sb[:, 0:512], d_ps)
                    else:
                        nc.vector.tensor_copy(o_sb[:, 512:1024], d_ps)
                        stsl = slice(st * 128, (st + 1) * 128)
                        nc.gpsimd.dma_start(out_d[stsl, :], o_sb)

                def pair(f, a, b):
                    def g():
                        f(*a)
                        f(*b)
                    return g
                chunks.append(pair(y1_chunk, (0, 0), (0, 1)))
                chunks.append(pair(y1_chunk, (1, 0), (1, 1)))
                chunks.append(pair(gin_chunk, (0, 0), (0, 1)))
                chunks.append(pair(gin_chunk, (1, 0), (1, 1)))
                for r in range(4):
                    for nw in range(2):
                        chunks.append(lambda r=r, nw=nw: down_chunk(r, nw))
                return chunks

            pending = []
            for sw in range(NSW):
                ssl = slice(sw * 512, (sw + 1) * 512)
                hd_res = []
                for hd in range(HPC):
                    adj_sb = adj_tiles[(sw, hd)]
                    sum_ps = ps2.tile([128, 2, 512], f32,
                                      name=f"sum{sw}_{hd}", tag="sum")
                    att_ps = ps2.tile([128, 2, 512], f32,
                                      name=f"att{sw}_{hd}", tag="att")
                    den_ps = ps2.tile([128, 512], f32,
                                      name=f"den{sw}_{hd}", tag="den", bufs=1)
                    em_tiles = {}
                    for pr in range(NPR + 1):
                        if pr < NPR:
                            em8 = strm.tile([128, 2, 512], fp8,
                                            name=f"em{sw}_{hd}_{pr}",
                                            tag="em", bufs=5)
                            ex = strm.tile([128, 2, 512], bf16,
                                           name=f"ex{sw}_{hd}_{pr}",
                                           tag="ex", bufs=4)
                            for i in range(2):
                                t = 2 * pr + i
                                tsl = slice(t * 128, (t + 1) * 128)
                                sc_ps = ps2.tile([128, 512], f32,
                                                 name=f"sc{sw}_{hd}_{pr}_{i}",
                                                 tag="mm", bufs=3)
                                nc.tensor.matmul(sc_ps, kT[:, hd, :, tsl],
                                                 qT[:, hd, :, ssl],
                                                 start=True, stop=True,
                                                 perf_mode=DR)
                                nc.scalar.activation(ex[:, i, :], sc_ps, AF.Exp,
                                                     scale=1.0 / 65536.0)
                            nc.vector.tensor_mul(em8, ex,
                                                 adj_sb[:, 2 * pr:2 * pr + 2, :])
                            em_tiles[pr] = em8
                        if pr >= 1:
                            p = pr - 1
                            em_p = em_tiles.pop(p)
                            first, last = p == 0, p == NPR - 1
                            for dc in range(2):
                                c0 = hd * D + dc * 128
                                for i in range(2):
                                    t = 2 * p + i
                                    nc.tensor.matmul(
                                        sum_ps[:, dc, :],
                                        h_all[:, t, c0:c0 + 128],
                                        adj_sb[:, t, :],
                                        start=(first and i == 0),
                                        stop=(last and i == 1))
                            nc.tensor.matmul(den_ps, ones2, em_p,
                                             start=first, stop=last,
                                             perf_mode=DR)
                            for dc in range(2):
                                c0 = hd * D + dc * 128
                                nc.tensor.matmul(
                                    att_ps[:, dc, :],
                                    h8_all[:, 2 * p:2 * p + 2, c0:c0 + 128],
                                    em_p, start=first, stop=last,
                                    perf_mode=DR)
                            if pending:
                                pending.pop(0)()

                    # evictions: sumT copy, recip (replicated den -> direct
                    # multiplier), attnT muls; all on vector
                    sumT = strm.tile([128, 2, 512], bf16, name=f"sumT{sw}_{hd}",
                                     tag="sumT", bufs=2)
                    nc.vector.tensor_copy(sumT, sum_ps)
                    rb_sb = strm.tile([128, 512], f32, name=f"rb{sw}_{hd}",
                                      tag="rb", bufs=2)
                    nc.vector.reciprocal_approx_fast(rb_sb, den_ps)
                    attnT = strm.tile([128, 2, 512], fp8, name=f"at{sw}_{hd}",
                                      tag="attnT", bufs=2)
                    for dc in range(2):
                        nc.vector.tensor_mul(attnT[:, dc, :],
                                             att_ps[:, dc, :], rb_sb)
                    hd_res.append((sumT, attnT))
                pending.extend(make_p3(sw, hd_res))
            for c in pending:
                c()

    nc.compile()
    return nc


def _prep_in_maps(x, adjacency, Wg, Wu, Wd, eps, alpha, Wq, Wk, W1, W2):
    f = lambda a: np.ascontiguousarray(a, dtype=np.float32)
    x, adjacency = f(x), f(adjacency)
    Wg, Wu, Wd, Wq, Wk, W1, W2 = map(f, (Wg, Wu, Wd, Wq, Wk, W1, W2))
    eps, alpha = f(eps), f(alpha)
    b16 = lambda a: np.ascontiguousarray(a).astype(BF16)
    f8 = lambda a: np.ascontiguousarray(a).astype(FP8)

    NST, NKC, NSW, NTC = S // 128, HID // 128, S // 512, S // 128
    # x blocks: [st, p, c, sl] = x[0, st*128+sl, c*128+p]
    xb = b16(x[0].reshape(NST, 128, NKC, 128).transpose(0, 3, 2, 1))

    W1a = W1[:, 0:D]
    W1b = W1[:, D:2 * D]
    W1c = W1[:, 2 * D:3 * D]
    W1d = W1[:, 3 * D:4 * D]

    in_maps = []
    for i in range(NCORES):
        hs = slice(i * HPC, (i + 1) * HPC)
        c0, c1 = i * LOC, (i + 1) * LOC
        # adj blocks: [sw, hd, p, tt, sl] = 16*adj[hd, sw*512+sl, tt*128+p]
        a = adjacency[0, hs].reshape(HPC, NSW, 512, NTC, 128)
        adjb = b16(16.0 * a.transpose(1, 0, 4, 3, 2))
        w1ac = np.stack([64.0 * ((1.0 + eps[h]) * W1a + W1c).T
                         for h in range(i * HPC, (i + 1) * HPC)])
        w1b = np.stack([32.0 * alpha[h] * W1b.T
                        for h in range(i * HPC, (i + 1) * HPC)])
        in_maps.append({
            "xb": xb,
            "wgb": b16(Wg[c0:c1].T.reshape(NKC, 128, LOC)),
            "wub": b16(Wu[c0:c1].T.reshape(NKC, 128, LOC)),
            "adjb": adjb,
            "wqT": f8(Wq[hs].transpose(0, 2, 1) * 8.0),
            "wkT": f8(Wk[hs].transpose(0, 2, 1) * 8.0),
            "w1acT": f8(w1ac),
            "w1bT": b16(w1b),
            "w1dT": f8(64.0 * W1d.T),
            "w2T": b16(W2.T),
            "wdT": b16(Wd[:, c0:c1].T),
        })
    return in_maps


def _run(inputs, trace=False, trace_kwargs=None):
    from concourse.bass_utils import run_bass_kernel_spmd

    if "nc" not in _CACHE:
        _CACHE["nc"] = _build_nc()
    nc = _CACHE["nc"]
    in_maps = _prep_in_maps(**inputs)
    res = run_bass_kernel_spmd(nc, in_maps, list(range(NCORES)),
                               trace=trace, **(trace_kwargs or {}))
    out = np.zeros((S, HID), np.float32)
    for r in res.results:
        out += r["out"]
    return out.reshape(B, S, HID), res


def kernel(**inputs) -> np.ndarray:
    out, _ = _run(inputs, trace=False)
    return out


# revision 10
# speedup vs baseline: 1.3307x; 1.0243x over previous
"""Trainium2 Bass kernel for nn_LlamaMLP_HalfwayGIN_MultiAggregration.

Sharding: 16 heads -> 8 cores (2 heads/core). Each core computes its two
heads' full pipeline plus the partial down-projection; host sums partials.

v2: fp8 DoubleRow for the attention-aggregation + denominator matmuls
(t-pairs, contraction 256), fp8 DR for the small y1 terms, contiguous
host-side DMA layouts (one 2MB DMA per adjacency stripe), sw-outer loop
with the GIN MLP + down-projection interleaved per s-window.

Scale folding (host):
  hT8/h8 = 8*h (fp8); wq8 = 8*Wq^T, wk8 = 8*Wk^T  -> scores psum = 4096*QK^T
  exp scale = 1/(4096*sqrt(D)) = 1/65536
  adj16 = 16*adjacency^T (bf16)  -> em8 = 16*em, sumT = 16*sum_raw
  w1b = 32*alpha*W1b^T (bf16: 512/16), w1ac8 = 64*((1+eps)W1a+W1c)^T (fp8:
  512/8 vs h8), w1d8 = 64*W1d^T (fp8: 512/8 vs attnT=8*attn), silu scale 1/512
"""

import math
import os
import numpy as np
import ml_dtypes

B, S, HID, NH, INTER = 1, 2048, 1024, 16, 4096
D = 256
NCORES = 8
HPC = NH // NCORES          # 2 heads per core
LOC = HPC * D               # 512 local intermediate dims
BF16 = ml_dtypes.bfloat16
FP8 = ml_dtypes.float8_e4m3

_CACHE = {}


def _build_nc():
    import concourse.mybir as mybir
    import concourse.tile as tile
    from concourse import bacc
    from concourse.masks import make_identity
    from contextlib import ExitStack

    f32 = mybir.dt.float32
    bf16 = mybir.dt.bfloat16
    fp8 = mybir.dt.float8e4
    AF = mybir.ActivationFunctionType
    DR = mybir.MatmulPerfMode.DoubleRow

    nc = bacc.Bacc("TRN2", target_bir_lowering=False, debug=False)

    NST = S // 128            # 16 s-tiles
    NSW = S // 512            # 4 s-windows
    NTC = S // 128            # 16 t-chunks
    NPR = NTC // 2            # 8 t-pairs
    NKC = HID // 128          # 8 k-chunks

    x_d = nc.dram_tensor("xb", [NST, 128, NKC, 128], bf16, kind="ExternalInput")
    wg_d = nc.dram_tensor("wgb", [NKC, 128, LOC], bf16, kind="ExternalInput")
    wu_d = nc.dram_tensor("wub", [NKC, 128, LOC], bf16, kind="ExternalInput")
    adj_d = nc.dram_tensor("adjb", [NSW, HPC, 128, NTC, 512], bf16,
                           kind="ExternalInput")
    wq_d = nc.dram_tensor("wqT", [HPC, D, D], fp8, kind="ExternalInput")
    wk_d = nc.dram_tensor("wkT", [HPC, D, D], fp8, kind="ExternalInput")
    w1ac_d = nc.dram_tensor("w1acT", [HPC, D, D], fp8, kind="ExternalInput")
    w1b_d = nc.dram_tensor("w1bT", [HPC, D, D], bf16, kind="ExternalInput")
    w1d_d = nc.dram_tensor("w1dT", [D, D], fp8, kind="ExternalInput")
    w2_d = nc.dram_tensor("w2T", [D, D], bf16, kind="ExternalInput")
    wd_d = nc.dram_tensor("wdT", [LOC, HID], bf16, kind="ExternalInput")
    out_d = nc.dram_tensor("out", [S, HID], bf16, kind="ExternalOutput")

    with ExitStack() as es:
        tc = es.enter_context(tile.TileContext(nc))

        persist = es.enter_context(tc.tile_pool(name="persist", bufs=1))
        h_all = persist.tile([128, NST, LOC], bf16, name="h_all")
        h8_all = persist.tile([128, NST, LOC], fp8, name="h8_all")
        hT8 = persist.tile([128, 2 * HPC, S], fp8, name="hT8")
        qT = persist.tile([128, HPC, 2, S], fp8, name="qT")
        kT = persist.tile([128, HPC, 2, S], fp8, name="kT")

        wpool = es.enter_context(tc.tile_pool(name="weights", bufs=1))
        wq_sb = wpool.tile([128, 2 * HPC, D], fp8, name="wq_sb")
        wk_sb = wpool.tile([128, 2 * HPC, D], fp8, name="wk_sb")
        w1ac_sb = wpool.tile([128, 2 * HPC, D], fp8, name="w1ac_sb")
        w1b_sb = wpool.tile([128, 2 * HPC, D], bf16, name="w1b_sb")
        w1d_sb = wpool.tile([128, 2, D], fp8, name="w1d_sb")
        w2_sb = wpool.tile([128, 2, D], bf16, name="w2_sb")
        wd_sb = wpool.tile([128, LOC // 128, HID], bf16, name="wd_sb")

        misc = es.enter_context(tc.tile_pool(name="misc", bufs=1))
        id_sb = misc.tile([128, 128], bf16, name="id_sb")
        ones2 = misc.tile([128, 2, 128], fp8, name="ones2")

        make_identity(nc, id_sb)
        nc.vector.memset(ones2, 1.0)

        adjpool = es.enter_context(tc.tile_pool(name="adj", bufs=1))

        # ---- phase 1: h = silu(x@WgT)*(x@WuT); hT8, h8 side copies ----
        with tc.tile_pool(name="xpool", bufs=1) as xpool, \
             tc.tile_pool(name="ps1", bufs=1, space="PSUM") as ps1, \
             tc.tile_pool(name="hstage", bufs=3) as hstage:
            x_sb = xpool.tile([128, NST, NKC, 128], bf16, name="x_sb")
            wg_sb = xpool.tile([128, NKC, LOC], bf16, name="wg_sb")
            wu_sb = xpool.tile([128, NKC, LOC], bf16, name="wu_sb")
            # wg/wu + first-half x interleaved on sync; rest of x on
            # gpsimd; small weights and adj stripes queue behind on sync so
            # phase-1 loads get the HBM bandwidth first
            nc.sync.dma_start(x_sb[:, 0], x_d[0])
            for c in range(NKC):
                nc.sync.dma_start(wg_sb[:, c, :], wg_d[c])
                nc.sync.dma_start(wu_sb[:, c, :], wu_d[c])
            for st in range(1, NST):
                nc.sync.dma_start(x_sb[:, st], x_d[st])
            nc.sync.dma_start(wq_sb, wq_d.rearrange("h (c p) e -> p (h c) e", p=128))
            nc.sync.dma_start(wk_sb, wk_d.rearrange("h (c p) e -> p (h c) e", p=128))
            nc.sync.dma_start(w1ac_sb, w1ac_d.rearrange("h (c p) o -> p (h c) o", p=128))
            nc.sync.dma_start(w1b_sb, w1b_d.rearrange("h (c p) o -> p (h c) o", p=128))
            nc.sync.dma_start(w1d_sb, w1d_d.rearrange("(c p) o -> p c o", p=128))
            nc.sync.dma_start(w2_sb, w2_d.rearrange("(c p) o -> p c o", p=128))
            nc.sync.dma_start(wd_sb, wd_d.rearrange("(c p) o -> p c o", p=128))
            adj_tiles = {}
            for sw in range(NSW):
                for hd in range(HPC):
                    a = adjpool.tile([128, NTC, 512], bf16,
                                     name=f"adj{sw}_{hd}", tag="adj", bufs=3)
                    nc.sync.dma_start(a, adj_d[sw, hd])
                    adj_tiles[(sw, hd)] = a

            def do_tr(st):
                # transpose s-tile st's four d-chunks (pipelined one behind)
                tr_ps = ps1.tile([128, 2 * HPC, 128], bf16, name=f"tr{st}",
                                 tag="tr", bufs=2)
                for j in range(2 * HPC):
                    col0 = j * 128
                    nc.tensor.transpose(tr_ps[:, j, :],
                                        h_all[:, st, col0:col0 + 128], id_sb)
                stsl = slice(st * 128, (st + 1) * 128)
                nc.vector.tensor_scalar_mul(hT8[:, :, stsl], tr_ps, 8.0)

            def do_qk(sw):
                ssl = slice(sw * 512, (sw + 1) * 512)
                for hd in range(HPC):
                    for w_sb, dstT in ((wq_sb, qT), (wk_sb, kT)):
                        for et in range(2):
                            ps = ps1.tile([128, 512], f32,
                                          name=f"qk{hd}_{et}_{sw}", tag="g",
                                          bufs=2)
                            nc.tensor.matmul(
                                ps,
                                w_sb[:, hd * 2:hd * 2 + 2, et * 128:(et + 1) * 128],
                                hT8[:, hd * 2:hd * 2 + 2, ssl],
                                start=True, stop=True, perf_mode=DR)
                            nc.vector.tensor_copy(dstT[:, hd, et, ssl], ps)

            for st in range(NST):
                g_ps = ps1.tile([128, LOC], f32, name=f"g{st}", tag="g", bufs=2)
                u_ps = ps1.tile([128, LOC], f32, name=f"u{st}", tag="u", bufs=2)
                for c in range(NKC):
                    lhsT = x_sb[:, st, c, :]
                    nc.tensor.matmul(g_ps, lhsT, wg_sb[:, c, :],
                                     start=(c == 0), stop=(c == NKC - 1))
                    nc.tensor.matmul(u_ps, lhsT, wu_sb[:, c, :],
                                     start=(c == 0), stop=(c == NKC - 1))
                if st >= 1:
                    do_tr(st - 1)
                sg = hstage.tile([128, LOC], bf16, name=f"sg{st}", tag="sg")
                nc.scalar.activation(sg, g_ps, AF.Silu)
                nc.vector.tensor_mul(h_all[:, st, :], sg, u_ps)
                nc.vector.tensor_scalar_mul(h8_all[:, st, :],
                                            h_all[:, st, :], 8.0)
                if st % 4 == 3 and st >= 7:
                    do_qk(st // 4 - 1)
            do_tr(NST - 1)
            do_qk(NSW - 1)

        # ---- phase 2+3 fused, sw-outer; p3 of window sw-1 interleaved
        # into window sw's attention pair loop ----
        with tc.tile_pool(name="stream", bufs=1) as strm, \
             tc.tile_pool(name="outp", bufs=2) as outp, \
             tc.tile_pool(name="ps2", bufs=1, space="PSUM") as ps2:

            def make_p3(sw, hd_res):
                """Phase-3 chunk closures for window sw (16 chunks)."""
                ssl = slice(sw * 512, (sw + 1) * 512)
                ginT = strm.tile([128, 2 * HPC, 512], bf16, name=f"gin{sw}",
                                 tag="gin", bufs=2)
                y1Ts = [strm.tile([128, 2, 512], bf16, name=f"y1{sw}_{hd}",
                                  tag=f"y1_{hd}", bufs=2) for hd in range(HPC)]
                chunks = []

                def y1_chunk(hd, ot):
                    sumT, attnT = hd_res[hd]
                    osl = slice(ot * 128, (ot + 1) * 128)
                    y1_ps = ps2.tile([128, 512], f32,
                                     name=f"y1p{sw}_{hd}_{ot}", tag="mm",
                                     bufs=3)
                    for dc in range(2):
                        nc.tensor.matmul(y1_ps, w1b_sb[:, hd * 2 + dc, osl],
                                         sumT[:, dc, :],
                                         start=(dc == 0), stop=False)
                    nc.tensor.matmul(y1_ps, w1ac_sb[:, hd * 2:hd * 2 + 2, osl],
                                     hT8[:, hd * 2:hd * 2 + 2, ssl],
                                     start=False, stop=False, perf_mode=DR)
                    nc.tensor.matmul(y1_ps, w1d_sb[:, :, osl], attnT,
                                     start=False, stop=True, perf_mode=DR)
                    nc.scalar.activation(y1Ts[hd][:, ot, :], y1_ps, AF.Silu,
                                         scale=1.0 / 512.0)

                def gin_chunk(hd, ot):
                    osl = slice(ot * 128, (ot + 1) * 128)
                    gin_ps = ps2.tile([128, 512], f32,
                                      name=f"ginp{sw}_{hd}_{ot}", tag="mm",
                                      bufs=3)
                    for dc in range(2):
                        nc.tensor.matmul(gin_ps, w2_sb[:, dc, osl],
                                         y1Ts[hd][:, dc, :],
                                         start=(dc == 0), stop=(dc == 1))
                    nc.scalar.activation(ginT[:, hd * 2 + ot, :], gin_ps,
                                         AF.Copy)

                o_sbs = {}

                def down_chunk(r, nw):
                    st = sw * 4 + r
                    rsl = slice(r * 128, (r + 1) * 128)
                    if nw == 0:
                        o_sbs[r] = outp.tile([128, HID], bf16, name=f"o{st}",
                                             tag="o")
                    o_sb = o_sbs[r]
                    d_ps = ps2.tile([128, 512], f32, name=f"d{st}_{nw}",
                                    tag="mm", bufs=3)
                    for j in range(LOC // 128):
                        nc.tensor.matmul(d_ps, ginT[:, j, rsl],
                                         wd_sb[:, j, nw * 512:(nw + 1) * 512],
                                         start=(j == 0),
                                         stop=(j == LOC // 128 - 1))
                    if nw == 0:
                        nc.vector.tensor_copy(o_sb[:, 0:512], d_ps)
                    else:
                        nc.vector.tensor_copy(o_sb[:, 512:1024], d_ps)
                        stsl = slice(st * 128, (st + 1) * 128)
                        nc.gpsimd.dma_start(out_d[stsl, :], o_sb)

                def pair(f, a, b):
                    def g():
                        f(*a)
                        f(*b)
                    return g
                chunks.append(pair(y1_chunk, (0, 0), (0, 1)))
                chunks.append(pair(y1_chunk, (1, 0), (1, 1)))
                chunks.append(pair(gin_chunk, (0, 0), (0, 1)))
                chunks.append(pair(gin_chunk, (1, 0), (1, 1)))
                for r in range(4):
                    for nw in range(2):
                        chunks.append(lambda r=r, nw=nw: down_chunk(r, nw))
                return chunks

            pending = []
            for sw in range(NSW):
                ssl = slice(sw * 512, (sw + 1) * 512)
                hd_res = []
                for hd in range(HPC):
                    adj_sb = adj_tiles[(sw, hd)]
                    sum_ps = ps2.tile([128, 2, 512], f32,
                                      name=f"sum{sw}_{hd}", tag="sum")
                    att_ps = ps2.tile([128, 2, 512], f32,
                                      name=f"att{sw}_{hd}", tag="att")
                    den_ps = ps2.tile([128, 512], f32,
                                      name=f"den{sw}_{hd}", tag="den", bufs=1)
                    em_tiles = {}
                    for pr in range(NPR + 1):
                        if pr < NPR:
                            em8 = strm.tile([128, 2, 512], fp8,
                                            name=f"em{sw}_{hd}_{pr}",
                                            tag="em", bufs=5)
                            ex = strm.tile([128, 2, 512], bf16,
                                           name=f"ex{sw}_{hd}_{pr}",
                                           tag="ex", bufs=4)
                            for i in range(2):
                                t = 2 * pr + i
                                tsl = slice(t * 128, (t + 1) * 128)
                                sc_ps = ps2.tile([128, 512], f32,
                                                 name=f"sc{sw}_{hd}_{pr}_{i}",
                                                 tag="mm", bufs=3)
                                nc.tensor.matmul(sc_ps, kT[:, hd, :, tsl],
                                                 qT[:, hd, :, ssl],
                                                 start=True, stop=True,
                                                 perf_mode=DR)
                                nc.scalar.activation(ex[:, i, :], sc_ps, AF.Exp,
                                                     scale=1.0 / 65536.0)
                            nc.vector.tensor_mul(em8, ex,
                                                 adj_sb[:, 2 * pr:2 * pr + 2, :])
                            em_tiles[pr] = em8
                        if pr >= 1:
                            p = pr - 1
                            em_p = em_tiles.pop(p)
                            first, last = p == 0, p == NPR - 1
                            for dc in range(2):
                                c0 = hd * D + dc * 128
                                for i in range(2):
                                    t = 2 * p + i
                                    nc.tensor.matmul(
                                        sum_ps[:, dc, :],
                                        h_all[:, t, c0:c0 + 128],
                                        adj_sb[:, t, :],
                                        start=(first and i == 0),
                                        stop=(last and i == 1))
                            nc.tensor.matmul(den_ps, ones2, em_p,
                                             start=first, stop=last,
                                             perf_mode=DR)
                            for dc in range(2):
                                c0 = hd * D + dc * 128
                                nc.tensor.matmul(
                                    att_ps[:, dc, :],
                                    h8_all[:, 2 * p:2 * p + 2, c0:c0 + 128],
                                    em_p, start=first, stop=last,
                                    perf_mode=DR)
                            if pending:
                                pending.pop(0)()

                    # evictions: sumT copy, recip (replicated den -> direct
                    # multiplier), attnT muls; all on vector
                    sumT = strm.tile([128, 2, 512], bf16, name=f"sumT{sw}_{hd}",
                                     tag="sumT", bufs=2)
                    nc.vector.tensor_copy(sumT, sum_ps)
                    rb_sb = strm.tile([128, 512], f32, name=f"rb{sw}_{hd}",
                                      tag="rb", bufs=2)
                    nc.vector.reciprocal_approx_fast(rb_sb, den_ps)
                    attnT = strm.tile([128, 2, 512], fp8, name=f"at{sw}_{hd}",
                                      tag="attnT", bufs=2)
                    for dc in range(2):
                        nc.vector.tensor_mul(attnT[:, dc, :],
                                             att_ps[:, dc, :], rb_sb)
                    hd_res.append((sumT, attnT))
                pending.extend(make_p3(sw, hd_res))
            for c in pending:
                c()

    nc.compile()
    return nc


def _prep_in_maps(x, adjacency, Wg, Wu, Wd, eps, alpha, Wq, Wk, W1, W2):
    f = lambda a: np.ascontiguousarray(a, dtype=np.float32)
    x, adjacency = f(x), f(adjacency)
    Wg, Wu, Wd, Wq, Wk, W1, W2 = map(f, (Wg, Wu, Wd, Wq, Wk, W1, W2))
    eps, alpha = f(eps), f(alpha)
    b16 = lambda a: np.ascontiguousarray(a).astype(BF16)
    f8 = lambda a: np.ascontiguousarray(a).astype(FP8)

    NST, NKC, NSW, NTC = S // 128, HID // 128, S // 512, S // 128
    # x blocks: [st, p, c, sl] = x[0, st*128+sl, c*128+p]
    xb = b16(x[0].reshape(NST, 128, NKC, 128).transpose(0, 3, 2, 1))

    W1a = W1[:, 0:D]
    W1b = W1[:, D:2 * D]
    W1c = W1[:, 2 * D:3 * D]
    W1d = W1[:, 3 * D:4 * D]

    in_maps = []
    for i in range(NCORES):
        hs = slice(i * HPC, (i + 1) * HPC)
        c0, c1 = i * LOC, (i + 1) * LOC
        # adj blocks: [sw, hd, p, tt, sl] = 16*adj[hd, sw*512+sl, tt*128+p]
        a = adjacency[0, hs].reshape(HPC, NSW, 512, NTC, 128)
        adjb = b16(16.0 * a.transpose(1, 0, 4, 3, 2))
        w1ac = np.stack([64.0 * ((1.0 + eps[h]) * W1a + W1c).T
                         for h in range(i * HPC, (i + 1) * HPC)])
        w1b = np.stack([32.0 * alpha[h] * W1b.T
                        for h in range(i * HPC, (i + 1) * HPC)])
        in_maps.append({
            "xb": xb,
            "wgb": b16(Wg[c0:c1].T.reshape(NKC, 128, LOC)),
            "wub": b16(Wu[c0:c1].T.reshape(NKC, 128, LOC)),
            "adjb": adjb,
            "wqT": f8(Wq[hs].transpose(0, 2, 1) * 8.0),
            "wkT": f8(Wk[hs].transpose(0, 2, 1) * 8.0),
            "w1acT": f8(w1ac),
            "w1bT": b16(w1b),
            "w1dT": f8(64.0 * W1d.T),
            "w2T": b16(W2.T),
            "wdT": b16(Wd[:, c0:c1].T),
        })
    return in_maps


def _run(inputs, trace=False, trace_kwargs=None):
    from concourse.bass_utils import run_bass_kernel_spmd

    if "nc" not in _CACHE:
        _CACHE["nc"] = _build_nc()
    nc = _CACHE["nc"]
    in_maps = _prep_in_maps(**inputs)
    res = run_bass_kernel_spmd(nc, in_maps, list(range(NCORES)),
                               trace=trace, **(trace_kwargs or {}))
    out = np.zeros((S, HID), np.float32)
    for r in res.results:
        out += np.asarray(r["out"], dtype=np.float32)
    return out.reshape(B, S, HID), res


def kernel(**inputs) -> np.ndarray:
    out, _ = _run(inputs, trace=False)
    return out


# revision 11
# speedup vs baseline: 1.3493x; 1.0140x over previous
"""Trainium2 Bass kernel for nn_LlamaMLP_HalfwayGIN_MultiAggregration.

Sharding: 16 heads -> 8 cores (2 heads/core). Each core computes its two
heads' full pipeline plus the partial down-projection; host sums partials.

v2: fp8 DoubleRow for the attention-aggregation + denominator matmuls
(t-pairs, contraction 256), fp8 DR for the small y1 terms, contiguous
host-side DMA layouts (one 2MB DMA per adjacency stripe), sw-outer loop
with the GIN MLP + down-projection interleaved per s-window.

Scale folding (host):
  hT8/h8 = 8*h (fp8); wq8 = 8*Wq^T, wk8 = 8*Wk^T  -> scores psum = 4096*QK^T
  exp scale = 1/(4096*sqrt(D)) = 1/65536
  adj16 = 16*adjacency^T (bf16)  -> em8 = 16*em, sumT = 16*sum_raw
  w1b = 32*alpha*W1b^T (bf16: 512/16), w1ac8 = 64*((1+eps)W1a+W1c)^T (fp8:
  512/8 vs h8), w1d8 = 64*W1d^T (fp8: 512/8 vs attnT=8*attn), silu scale 1/512
"""

import math
import os
import numpy as np
import ml_dtypes

B, S, HID, NH, INTER = 1, 2048, 1024, 16, 4096
D = 256
NCORES = 8
HPC = NH // NCORES          # 2 heads per core
LOC = HPC * D               # 512 local intermediate dims
BF16 = ml_dtypes.bfloat16
FP8 = ml_dtypes.float8_e4m3

_CACHE = {}


def _build_nc():
    import concourse.mybir as mybir
    import concourse.tile as tile
    from concourse import bacc
    from concourse.masks import make_identity
    from contextlib import ExitStack

    f32 = mybir.dt.float32
    bf16 = mybir.dt.bfloat16
    fp8 = mybir.dt.float8e4
    AF = mybir.ActivationFunctionType
    DR = mybir.MatmulPerfMode.DoubleRow

    nc = bacc.Bacc("TRN2", target_bir_lowering=False, debug=False)

    NST = S // 128            # 16 s-tiles
    NSW = S // 512            # 4 s-windows
    NTC = S // 128            # 16 t-chunks
    NPR = NTC // 2            # 8 t-pairs
    NKC = HID // 128          # 8 k-chunks

    x_d = nc.dram_tensor("xb", [NST, 128, NKC, 128], bf16, kind="ExternalInput")
    wg_d = nc.dram_tensor("wgb", [NKC, 128, LOC], bf16, kind="ExternalInput")
    wu_d = nc.dram_tensor("wub", [NKC, 128, LOC], bf16, kind="ExternalInput")
    adj_d = nc.dram_tensor("adjb", [NSW, HPC, 128, NTC, 512], bf16,
                           kind="ExternalInput")
    wq_d = nc.dram_tensor("wqT", [HPC, D, D], fp8, kind="ExternalInput")
    wk_d = nc.dram_tensor("wkT", [HPC, D, D], fp8, kind="ExternalInput")
    w1ac_d = nc.dram_tensor("w1acT", [HPC, D, D], fp8, kind="ExternalInput")
    w1b_d = nc.dram_tensor("w1bT", [HPC, D, D], bf16, kind="ExternalInput")
    w1d_d = nc.dram_tensor("w1dT", [D, D], fp8, kind="ExternalInput")
    w2_d = nc.dram_tensor("w2T", [D, D], bf16, kind="ExternalInput")
    wd_d = nc.dram_tensor("wdT", [LOC, HID], bf16, kind="ExternalInput")
    out_d = nc.dram_tensor("out", [S, HID], bf16, kind="ExternalOutput")

    with ExitStack() as es:
        tc = es.enter_context(tile.TileContext(nc))

        persist = es.enter_context(tc.tile_pool(name="persist", bufs=1))
        h_all = persist.tile([128, NST, LOC], bf16, name="h_all")
        h8_all = persist.tile([128, NST, LOC], fp8, name="h8_all")
        hT8 = persist.tile([128, 2 * HPC, S], fp8, name="hT8")
        qT = persist.tile([128, HPC, 2, S], fp8, name="qT")
        kT = persist.tile([128, HPC, 2, S], fp8, name="kT")

        wpool = es.enter_context(tc.tile_pool(name="weights", bufs=1))
        wq_sb = wpool.tile([128, 2 * HPC, D], fp8, name="wq_sb")
        wk_sb = wpool.tile([128, 2 * HPC, D], fp8, name="wk_sb")
        w1ac_sb = wpool.tile([128, 2 * HPC, D], fp8, name="w1ac_sb")
        w1b_sb = wpool.tile([128, 2 * HPC, D], bf16, name="w1b_sb")
        w1d_sb = wpool.tile([128, 2, D], fp8, name="w1d_sb")
        w2_sb = wpool.tile([128, 2, D], bf16, name="w2_sb")
        wd_sb = wpool.tile([128, LOC // 128, HID], bf16, name="wd_sb")

        misc = es.enter_context(tc.tile_pool(name="misc", bufs=1))
        id_sb = misc.tile([128, 128], bf16, name="id_sb")
        ones2 = misc.tile([128, 2, 128], fp8, name="ones2")

        make_identity(nc, id_sb)
        nc.vector.memset(ones2, 1.0)

        adjpool = es.enter_context(tc.tile_pool(name="adj", bufs=1))

        # ---- phase 1: h = silu(x@WgT)*(x@WuT); hT8, h8 side copies ----
        with tc.tile_pool(name="xpool", bufs=1) as xpool, \
             tc.tile_pool(name="ps1", bufs=1, space="PSUM") as ps1, \
             tc.tile_pool(name="hstage", bufs=3) as hstage:
            x_sb = xpool.tile([128, NST, NKC, 128], bf16, name="x_sb")
            wg_sb = xpool.tile([128, NKC, LOC], bf16, name="wg_sb")
            wu_sb = xpool.tile([128, NKC, LOC], bf16, name="wu_sb")
            # wg/wu + first-half x interleaved on sync; rest of x on
            # gpsimd; small weights and adj stripes queue behind on sync so
            # phase-1 loads get the HBM bandwidth first
            nc.sync.dma_start(x_sb[:, 0], x_d[0])
            for c in range(NKC):
                nc.sync.dma_start(wg_sb[:, c, :], wg_d[c])
                nc.sync.dma_start(wu_sb[:, c, :], wu_d[c])
            for st in range(1, NST):
                nc.sync.dma_start(x_sb[:, st], x_d[st])
            nc.sync.dma_start(wq_sb, wq_d.rearrange("h (c p) e -> p (h c) e", p=128))
            nc.sync.dma_start(wk_sb, wk_d.rearrange("h (c p) e -> p (h c) e", p=128))
            nc.sync.dma_start(w1ac_sb, w1ac_d.rearrange("h (c p) o -> p (h c) o", p=128))
            nc.sync.dma_start(w1b_sb, w1b_d.rearrange("h (c p) o -> p (h c) o", p=128))
            nc.sync.dma_start(w1d_sb, w1d_d.rearrange("(c p) o -> p c o", p=128))
            nc.sync.dma_start(w2_sb, w2_d.rearrange("(c p) o -> p c o", p=128))
            nc.sync.dma_start(wd_sb, wd_d.rearrange("(c p) o -> p c o", p=128))
            adj_tiles = {}
            for sw in range(NSW):
                for hd in range(HPC):
                    a = adjpool.tile([128, NTC, 512], bf16,
                                     name=f"adj{sw}_{hd}", tag="adj", bufs=3)
                    nc.sync.dma_start(a, adj_d[sw, hd])
                    adj_tiles[(sw, hd)] = a

            # PE warm-up: keep the ramp going while the first x/w chunks
            # stream in (dummy transposes of the identity tile)
            warm = ps1.tile([128, 128], bf16, name="warm", tag="tr", bufs=2)
            for _ in range(24):
                nc.tensor.transpose(warm, id_sb, id_sb)

            def do_tr(st):
                # transpose s-tile st's four d-chunks (pipelined one behind)
                tr_ps = ps1.tile([128, 2 * HPC, 128], bf16, name=f"tr{st}",
                                 tag="tr", bufs=2)
                for j in range(2 * HPC):
                    col0 = j * 128
                    nc.tensor.transpose(tr_ps[:, j, :],
                                        h_all[:, st, col0:col0 + 128], id_sb)
                stsl = slice(st * 128, (st + 1) * 128)
                nc.vector.tensor_scalar_mul(hT8[:, :, stsl], tr_ps, 8.0)

            def do_qk(sw):
                ssl = slice(sw * 512, (sw + 1) * 512)
                for hd in range(HPC):
                    for w_sb, dstT in ((wq_sb, qT), (wk_sb, kT)):
                        for et in range(2):
                            ps = ps1.tile([128, 512], f32,
                                          name=f"qk{hd}_{et}_{sw}", tag="g",
                                          bufs=2)
                            nc.tensor.matmul(
                                ps,
                                w_sb[:, hd * 2:hd * 2 + 2, et * 128:(et + 1) * 128],
                                hT8[:, hd * 2:hd * 2 + 2, ssl],
                                start=True, stop=True, perf_mode=DR)
                            nc.vector.tensor_copy(dstT[:, hd, et, ssl], ps)

            for st in range(NST):
                g_ps = ps1.tile([128, LOC], f32, name=f"g{st}", tag="g", bufs=2)
                u_ps = ps1.tile([128, LOC], f32, name=f"u{st}", tag="u", bufs=2)
                for c in range(NKC):
                    lhsT = x_sb[:, st, c, :]
                    nc.tensor.matmul(g_ps, lhsT, wg_sb[:, c, :],
                                     start=(c == 0), stop=(c == NKC - 1))
                    nc.tensor.matmul(u_ps, lhsT, wu_sb[:, c, :],
                                     start=(c == 0), stop=(c == NKC - 1))
                if st >= 1:
                    do_tr(st - 1)
                sg = hstage.tile([128, LOC], bf16, name=f"sg{st}", tag="sg")
                nc.scalar.activation(sg, g_ps, AF.Silu)
                nc.vector.tensor_mul(h_all[:, st, :], sg, u_ps)
                nc.vector.tensor_scalar_mul(h8_all[:, st, :],
                                            h_all[:, st, :], 8.0)
                if st % 4 == 3 and st >= 7:
                    do_qk(st // 4 - 1)
            do_tr(NST - 1)
            do_qk(NSW - 1)

        # ---- phase 2+3 fused, sw-outer; p3 of window sw-1 interleaved
        # into window sw's attention pair loop ----
        with tc.tile_pool(name="stream", bufs=1) as strm, \
             tc.tile_pool(name="outp", bufs=2) as outp, \
             tc.tile_pool(name="ps2", bufs=1, space="PSUM") as ps2:

            def make_p3(sw, hd_res):
                """Phase-3 chunk closures for window sw (16 chunks)."""
                ssl = slice(sw * 512, (sw + 1) * 512)
                ginT = strm.tile([128, 2 * HPC, 512], bf16, name=f"gin{sw}",
                                 tag="gin", bufs=2)
                y1Ts = [strm.tile([128, 2, 512], bf16, name=f"y1{sw}_{hd}",
                                  tag=f"y1_{hd}", bufs=2) for hd in range(HPC)]
                chunks = []

                def y1_chunk(hd, ot):
                    sumT, attnT = hd_res[hd]
                    osl = slice(ot * 128, (ot + 1) * 128)
                    y1_ps = ps2.tile([128, 512], f32,
                                     name=f"y1p{sw}_{hd}_{ot}", tag="mm",
                                     bufs=3)
                    for dc in range(2):
                        nc.tensor.matmul(y1_ps, w1b_sb[:, hd * 2 + dc, osl],
                                         sumT[:, dc, :],
                                         start=(dc == 0), stop=False)
                    nc.tensor.matmul(y1_ps, w1ac_sb[:, hd * 2:hd * 2 + 2, osl],
                                     hT8[:, hd * 2:hd * 2 + 2, ssl],
                                     start=False, stop=False, perf_mode=DR)
                    nc.tensor.matmul(y1_ps, w1d_sb[:, :, osl], attnT,
                                     start=False, stop=True, perf_mode=DR)
                    nc.scalar.activation(y1Ts[hd][:, ot, :], y1_ps, AF.Silu,
                                         scale=1.0 / 512.0)

                def gin_chunk(hd, ot):
                    osl = slice(ot * 128, (ot + 1) * 128)
                    gin_ps = ps2.tile([128, 512], f32,
                                      name=f"ginp{sw}_{hd}_{ot}", tag="mm",
                                      bufs=3)
                    for dc in range(2):
                        nc.tensor.matmul(gin_ps, w2_sb[:, dc, osl],
                                         y1Ts[hd][:, dc, :],
                                         start=(dc == 0), stop=(dc == 1))
                    nc.scalar.activation(ginT[:, hd * 2 + ot, :], gin_ps,
                                         AF.Copy)

                o_sbs = {}

                def down_chunk(r, nw):
                    st = sw * 4 + r
                    rsl = slice(r * 128, (r + 1) * 128)
                    if nw == 0:
                        o_sbs[r] = outp.tile([128, HID], bf16, name=f"o{st}",
                                             tag="o")
                    o_sb = o_sbs[r]
                    d_ps = ps2.tile([128, 512], f32, name=f"d{st}_{nw}",
                                    tag="mm", bufs=3)
                    for j in range(LOC // 128):
                        nc.tensor.matmul(d_ps, ginT[:, j, rsl],
                                         wd_sb[:, j, nw * 512:(nw + 1) * 512],
                                         start=(j == 0),
                                         stop=(j == LOC // 128 - 1))
                    if nw == 0:
                        nc.vector.tensor_copy(o_sb[:, 0:512], d_ps)
                    else:
                        nc.vector.tensor_copy(o_sb[:, 512:1024], d_ps)
                        stsl = slice(st * 128, (st + 1) * 128)
                        nc.gpsimd.dma_start(out_d[stsl, :], o_sb)

                def pair(f, a, b):
                    def g():
                        f(*a)
                        f(*b)
                    return g
                chunks.append(pair(y1_chunk, (0, 0), (0, 1)))
                chunks.append(pair(y1_chunk, (1, 0), (1, 1)))
                chunks.append(pair(gin_chunk, (0, 0), (0, 1)))
                chunks.append(pair(gin_chunk, (1, 0), (1, 1)))
                for r in range(4):
                    for nw in range(2):
                        chunks.append(lambda r=r, nw=nw: down_chunk(r, nw))
                return chunks

            pending = []
            for sw in range(NSW):
                ssl = slice(sw * 512, (sw + 1) * 512)
                hd_res = []
                for hd in range(HPC):
                    adj_sb = adj_tiles[(sw, hd)]
                    sum_ps = ps2.tile([128, 2, 512], f32,
                                      name=f"sum{sw}_{hd}", tag="sum")
                    att_ps = ps2.tile([128, 2, 512], f32,
                                      name=f"att{sw}_{hd}", tag="att")
                    den_ps = ps2.tile([128, 512], f32,
                                      name=f"den{sw}_{hd}", tag="den", bufs=1)
                    em_tiles = {}
                    for pr in range(NPR + 1):
                        if pr < NPR:
                            em8 = strm.tile([128, 2, 512], fp8,
                                            name=f"em{sw}_{hd}_{pr}",
                                            tag="em", bufs=5)
                            ex = strm.tile([128, 2, 512], bf16,
                                           name=f"ex{sw}_{hd}_{pr}",
                                           tag="ex", bufs=4)
                            for i in range(2):
                                t = 2 * pr + i
                                tsl = slice(t * 128, (t + 1) * 128)
                                sc_ps = ps2.tile([128, 512], f32,
                                                 name=f"sc{sw}_{hd}_{pr}_{i}",
                                                 tag="mm", bufs=3)
                                nc.tensor.matmul(sc_ps, kT[:, hd, :, tsl],
                                                 qT[:, hd, :, ssl],
                                                 start=True, stop=True,
                                                 perf_mode=DR)
                                nc.scalar.activation(ex[:, i, :], sc_ps, AF.Exp,
                                                     scale=1.0 / 65536.0)
                            nc.vector.tensor_mul(em8, ex,
                                                 adj_sb[:, 2 * pr:2 * pr + 2, :])
                            em_tiles[pr] = em8
                        if pr >= 1:
                            p = pr - 1
                            em_p = em_tiles.pop(p)
                            first, last = p == 0, p == NPR - 1
                            for dc in range(2):
                                c0 = hd * D + dc * 128
                                for i in range(2):
                                    t = 2 * p + i
                                    nc.tensor.matmul(
                                        sum_ps[:, dc, :],
                                        h_all[:, t, c0:c0 + 128],
                                        adj_sb[:, t, :],
                                        start=(first and i == 0),
                                        stop=(last and i == 1))
                            nc.tensor.matmul(den_ps, ones2, em_p,
                                             start=first, stop=last,
                                             perf_mode=DR)
                            for dc in range(2):
                                c0 = hd * D + dc * 128
                                nc.tensor.matmul(
                                    att_ps[:, dc, :],
                                    h8_all[:, 2 * p:2 * p + 2, c0:c0 + 128],
                                    em_p, start=first, stop=last,
                                    perf_mode=DR)
                            if pending:
                                pending.pop(0)()

                    # evictions: sumT copy, recip (replicated den -> direct
                    # multiplier), attnT muls; all on vector
                    sumT = strm.tile([128, 2, 512], bf16, name=f"sumT{sw}_{hd}",
                                     tag="sumT", bufs=2)
                    nc.vector.tensor_copy(sumT, sum_ps)
                    rb_sb = strm.tile([128, 512], f32, name=f"rb{sw}_{hd}",
                                      tag="rb", bufs=2)
                    nc.vector.reciprocal_approx_fast(rb_sb, den_ps)
                    attnT = strm.tile([128, 2, 512], fp8, name=f"at{sw}_{hd}",
                                      tag="attnT", bufs=2)
                    for dc in range(2):
                        nc.vector.tensor_mul(attnT[:, dc, :],
                                             att_ps[:, dc, :], rb_sb)
                    hd_res.append((sumT, attnT))
                pending.extend(make_p3(sw, hd_res))
            for c in pending:
                c()

    nc.compile()
    return nc


def _prep_in_maps(x, adjacency, Wg, Wu, Wd, eps, alpha, Wq, Wk, W1, W2):
    f = lambda a: np.ascontiguousarray(a, dtype=np.float32)
    x, adjacency = f(x), f(adjacency)
    Wg, Wu, Wd, Wq, Wk, W1, W2 = map(f, (Wg, Wu, Wd, Wq, Wk, W1, W2))
    eps, alpha = f(eps), f(alpha)
    b16 = lambda a: np.ascontiguousarray(a).astype(BF16)
    f8 = lambda a: np.ascontiguousarray(a).astype(FP8)

    NST, NKC, NSW, NTC = S // 128, HID // 128, S // 512, S // 128
    # x blocks: [st, p, c, sl] = x[0, st*128+sl, c*128+p]
    xb = b16(x[0].reshape(NST, 128, NKC, 128).transpose(0, 3, 2, 1))

    W1a = W1[:, 0:D]
    W1b = W1[:, D:2 * D]
    W1c = W1[:, 2 * D:3 * D]
    W1d = W1[:, 3 * D:4 * D]

    in_maps = []
    for i in range(NCORES):
        hs = slice(i * HPC, (i + 1) * HPC)
        c0, c1 = i * LOC, (i + 1) * LOC
        # adj blocks: [sw, hd, p, tt, sl] = 16*adj[hd, sw*512+sl, tt*128+p]
        a = adjacency[0, hs].reshape(HPC, NSW, 512, NTC, 128)
        adjb = b16(16.0 * a.transpose(1, 0, 4, 3, 2))
        w1ac = np.stack([64.0 * ((1.0 + eps[h]) * W1a + W1c).T
                         for h in range(i * HPC, (i + 1) * HPC)])
        w1b = np.stack([32.0 * alpha[h] * W1b.T
                        for h in range(i * HPC, (i + 1) * HPC)])
        in_maps.append({
            "xb": xb,
            "wgb": b16(Wg[c0:c1].T.reshape(NKC, 128, LOC)),
            "wub": b16(Wu[c0:c1].T.reshape(NKC, 128, LOC)),
            "adjb": adjb,
            "wqT": f8(Wq[hs].transpose(0, 2, 1) * 8.0),
            "wkT": f8(Wk[hs].transpose(0, 2, 1) * 8.0),
            "w1acT": f8(w1ac),
            "w1bT": b16(w1b),
            "w1dT": f8(64.0 * W1d.T),
            "w2T": b16(W2.T),
            "wdT": b16(Wd[:, c0:c1].T),
        })
    return in_maps


def _run(inputs, trace=False, trace_kwargs=None):
    from concourse.bass_utils import run_bass_kernel_spmd

    if "nc" not in _CACHE:
        _CACHE["nc"] = _build_nc()
    nc = _CACHE["nc"]
    in_maps = _prep_in_maps(**inputs)
    res = run_bass_kernel_spmd(nc, in_maps, list(range(NCORES)),
                               trace=trace, **(trace_kwargs or {}))
    out = np.zeros((S, HID), np.float32)
    for r in res.results:
        out += np.asarray(r["out"], dtype=np.float32)
    return out.reshape(B, S, HID), res


def kernel(**inputs) -> np.ndarray:
    out, _ = _run(inputs, trace=False)
    return out
